# revision 1
# baseline (speedup 1.0000x reference)
"""Trainium2 Bass kernel for nn_MetaLearner (dual-branch GCN + PPMI meta-learner).

Strategy
--------
Host side: the edge-list GCN propagation is folded into a dense normalized
adjacency matmul:  gcn_prop(h) = A' @ h  with  A'[d,s] = norm[d]*norm[s]*#edges(s->d),
which makes both branches identical dense pipelines:

  X    = feats @ [W1L | W1G]        (W1x = w1x @ tao_1_x, folded on host)
  H1b  = relu(M_b @ X_b + b1_b)     M_L = A', M_G = PPMI
  Y_b  = H1b @ W2b                  (W2b = w2b @ tao_2_b)
  H2b  = relu(M_b @ Y_b + b2_b)
  a0   = sigmoid(concat(H2L,H2G) @ (W_a[:,0]-W_a[:,1]))   (softmax of 2 = sigmoid of diff)
  Z    = H2G + a0*(H2L - H2G);  out = Z @ W_c + b_c

Device side, per core (rows split 1024-per-core across 8 cores; A'/PPMI passed
per-core pre-transposed K-major; X computed replicated since it is cheap):

  pass 1:  X chunks (stage 1) fused with prop1-G (PPMI streamed); X_L kept
           resident;  Y_G -> AllGather-G
  pass 2:  prop1-L fully from SBUF (X_L resident, A' streamed in and kept
           for prop2-L); the gather-G latency hides under this span;
           Y_L -> AllGather-L
  prop2 :  G chunks lead (their PPMI stream executes while gather-L flies),
           then 1:1 G/L interleave so the stream hides under compute, ending
           in a compute-only L tail; per-m-half pipelined attention+classifier.

All heavy matmuls run in fp16 (PPMI pre-scaled by N on the host, un-scaled in
the activation, to stay clear of fp16 subnormals) with fp32 PSUM accumulation.
Propagation outputs live transposed (features on partitions) so biases are
per-partition activation scalars.
"""

import sys

sys.path.insert(0, "/opt/trn_rl_repo")

import numpy as np

import concourse.bacc as bacc
import concourse.mybir as mybir
import concourse.tile as tile
from concourse.bass_utils import run_bass_kernel_spmd

N = 8192
D_IN = 512
D_H = 256
D_O = 128
N_CLS = 8
CORES = 8
M_LOC = N // CORES          # 1024 rows per core
SK = N // 128               # 64 contraction chunks of 128
MB = M_LOC // 128           # 8 local row blocks
KC = D_IN // 128            # 4 k-chunks of input features
NB = D_H // 128             # 2 n-blocks of hidden features
F2 = 512                    # matmul free-dim slice
NH = M_LOC // F2            # 2 free-dim halves of the local rows

HALF = mybir.dt.float16
F32 = mybir.dt.float32
AF = mybir.ActivationFunctionType

_CACHE = {}


def _build(collectives: bool = True):
    nc = bacc.Bacc("TRN2", target_bir_lowering=False, debug=False, num_devices=CORES)

    ftT_d = nc.dram_tensor("ftT", [D_IN, N], HALF, kind="ExternalInput")
    wb_d = nc.dram_tensor("w_both", [D_IN, 2 * D_H], HALF, kind="ExternalInput")
    w2lg_d = nc.dram_tensor("w2lg", [2, D_H, D_O], HALF, kind="ExternalInput")
    a_d = nc.dram_tensor("a_t", [N, M_LOC], HALF, kind="ExternalInput")
    p_d = nc.dram_tensor("p_t", [N, M_LOC], HALF, kind="ExternalInput")
    # biases packed [128, 7] f32: 0-1 b1, 2-3 b1g, 4 b2, 5 b2g, 6 b_c (rows 0-7)
    bias_d = nc.dram_tensor("biases", [128, 7], F32, kind="ExternalInput")
    # wadc packed [128, 10] fp16: 0 wad_L, 1 wad_G, 2-9 W_c
    wadc_d = nc.dram_tensor("wadc", [128, 10], HALF, kind="ExternalInput")
    out_d = nc.dram_tensor("outT", [N_CLS, M_LOC], F32, kind="ExternalOutput")

    ftT_v = ftT_d[:].rearrange("(kc p) (sb sl) -> p kc sb sl", p=128, sl=128)
    a_v = a_d[:].rearrange("(c p) m -> p c m", p=128)
    p_v = p_d[:].rearrange("(c p) m -> p c m", p=128)

    with tile.TileContext(nc) as tc:
        PREF = 12  # PPMI chunks prefetched ahead
        with (
            tc.tile_pool(name="const", bufs=1) as cpool,
            tc.tile_pool(name="a_res", bufs=SK) as a_pool,
            tc.tile_pool(name="p_strm", bufs=PREF) as p_pool,
            tc.tile_pool(name="ystage", bufs=2) as y_pool,
            tc.tile_pool(name="dram", bufs=1, space="DRAM") as dram,
        ):
            # ---- constants (gpsimd DMA queue: keep the SP queue free for the
            # big streams; wb first since stage 1 needs it immediately) ----
            wb_s = cpool.tile([128, KC, 2 * D_H], HALF, name="wb_s")
            nc.gpsimd.dma_start(wb_s[:], wb_d[:].rearrange("(kc p) m -> p kc m", p=128))
            w2_s = cpool.tile([128, 2, NB, D_O], HALF, name="w2_s")
            nc.gpsimd.dma_start(w2_s[:], w2lg_d[:].rearrange("b (c p) m -> p b c m", p=128))
            bias_s = cpool.tile([128, 7], F32, name="bias_s")
            nc.gpsimd.dma_start(bias_s[:], bias_d[:])
            wadc_s = cpool.tile([128, 10], HALF, name="wadc_s")
            nc.gpsimd.dma_start(wadc_s[:], wadc_d[:])
            ones_s = cpool.tile([1, 128], HALF, name="ones_s")
            nc.gpsimd.memset(ones_s[:], 1.0)
            # warm the sigmoid table set now (relu/copy are fillers in every
            # set, so no further ACT table switches happen mid-kernel)
            sig_warm = cpool.tile([1, 8], HALF, name="sig_warm")
            nc.scalar.activation(sig_warm[:], ones_s[:1, 0:8], AF.Sigmoid)

            a_tiles = [a_pool.tile([128, M_LOC], HALF, name=f"a{s}", tag="a") for s in range(SK)]

            yl_bounce = dram.tile([M_LOC, D_O], HALF, name="yl_bounce")
            yg_bounce = dram.tile([M_LOC, D_O], HALF, name="yg_bounce")
            yl_all = dram.tile([N, D_O], HALF, addr_space="Shared", name="yl_all")
            yg_all = dram.tile([N, D_O], HALF, addr_space="Shared", name="yg_all")

            h1_ctx = tc.tile_pool(name="h1", bufs=1)
            h1_pool = h1_ctx.__enter__()
            xl_ctx = tc.tile_pool(name="xl_res", bufs=SK)
            xl_pool = xl_ctx.__enter__()
            pg_tiles = {}

            def load_p(idx):
                p_s = p_pool.tile([128, M_LOC], HALF, name=f"p1_{idx}", tag="p")
                nc.sync.dma_start(p_s[:], p_v[:, idx, :])
                pg_tiles[idx] = p_s

            # ===== pass 1: X = feats@[W1L|W1G] fused with prop1-G (PPMI stream);
            # X_L stored resident, A' prefetched for pass 2 =====
            with (
                tc.tile_pool(name="ft", bufs=4) as ft_pool,
                tc.tile_pool(name="xgs", bufs=4) as xg_pool,
                tc.tile_pool(name="ps_x", bufs=2, space="PSUM") as psx_pool,
                tc.tile_pool(name="ps_G", bufs=1, space="PSUM") as psg_pool,
            ):
                psum_G = [psg_pool.tile([128, M_LOC], F32, name=f"psg{t}", tag=f"psg{t}") for t in range(NB)]
                xl_tiles = [xl_pool.tile([128, D_H], HALF, name=f"xl{s}", tag="xl") for s in range(SK)]

                xg_prev = None
                for s in range(SK):
                    ft_s = ft_pool.tile([128, KC, 128], HALF, name=f"ft{s}", tag="ft")
                    nc.sync.dma_start(ft_s[:], ftT_v[:, :, s, :])
                    load_p(s)
                    psx = psx_pool.tile([128, 2 * D_H], F32, name=f"psx{s}", tag="psx")
                    for k in range(KC):
                        nc.tensor.matmul(
                            psx[:], ft_s[:, k, :], wb_s[:, k, :],
                            start=(k == 0), stop=(k == KC - 1),
                        )
                    xg_s = xg_pool.tile([128, D_H], HALF, name=f"xg{s}", tag="xg")
                    nc.vector.tensor_copy(xg_s[:], psx[:, D_H:2 * D_H])
                    nc.vector.tensor_copy(xl_tiles[s][:], psx[:, 0:D_H])
                    # prop1-G for the previous chunk (software pipelining: PE has
                    # stage-1 work while DVE copies this chunk)
                    if xg_prev is not None:
                        sp, xp = xg_prev
                        pp = pg_tiles.pop(sp)
                        for t in range(NB):
                            for h in range(NH):
                                nc.tensor.matmul(
                                    psum_G[t][:, h * F2:(h + 1) * F2],
                                    xp[:, t * 128:(t + 1) * 128],
                                    pp[:, h * F2:(h + 1) * F2],
                                    start=(sp == 0), stop=(sp == SK - 1),
                                )
                    xg_prev = (s, xg_s)
                sp, xp = xg_prev
                pp = pg_tiles.pop(sp)
                for t in range(NB):
                    for h in range(NH):
                        nc.tensor.matmul(
                            psum_G[t][:, h * F2:(h + 1) * F2],
                            xp[:, t * 128:(t + 1) * 128],
                            pp[:, h * F2:(h + 1) * F2],
                            start=(sp == 0), stop=(sp == SK - 1),
                        )
                h1g = [h1_pool.tile([128, M_LOC], HALF, name=f"h1g{t}", tag=f"h1g{t}") for t in range(NB)]
                for t in range(NB):
                    nc.scalar.activation(h1g[t][:], psum_G[t][:], AF.Relu, bias=bias_s[:, 2 + t:3 + t], scale=1.0 / N)

            # Y_G = H1_G @ W2G (row-major out), bounce + AllGather-G
            with tc.tile_pool(name="ps_y2", bufs=2, space="PSUM") as psy_pool:
                yst = y_pool.tile([128, MB, D_O], HALF, name="ygst", tag="yst")
                for mb in range(MB):
                    psy = psy_pool.tile([128, D_O], F32, name=f"psyg{mb}", tag="psy")
                    for t in range(NB):
                        nc.tensor.matmul(
                            psy[:], h1g[t][:, mb * 128:(mb + 1) * 128], w2_s[:, 1, t, :],
                            start=(t == 0), stop=(t == NB - 1),
                        )
                    nc.vector.tensor_copy(yst[:, mb, :], psy[:])
                for mb in range(MB):
                    nc.sync.dma_start(yg_bounce[mb * 128:(mb + 1) * 128, :], yst[:, mb, :])
            if collectives:
                nc.gpsimd.collective_compute(
                    "AllGather", mybir.AluOpType.bypass,
                    ins=[yg_bounce.opt()], outs=[yg_all.opt()],
                    replica_groups=[list(range(CORES))],
                )
            else:
                nc.sync.dma_start(yg_all[0:M_LOC, :], yg_bounce[:])

            # ===== pass 2: prop1-L, fully resident (X_L and A' in SBUF) — the
            # gather-G latency hides under this compute-only span =====
            with tc.tile_pool(name="ps_L", bufs=1, space="PSUM") as psl_pool:
                psum_L = [psl_pool.tile([128, M_LOC], F32, name=f"psl{t}", tag=f"psl{t}") for t in range(NB)]
                for s in range(4):
                    nc.sync.dma_start(a_tiles[s][:], a_v[:, s, :])
                for s in range(SK):
                    if s + 4 < SK:
                        nc.sync.dma_start(a_tiles[s + 4][:], a_v[:, s + 4, :])
                    for t in range(NB):
                        for h in range(NH):
                            nc.tensor.matmul(
                                psum_L[t][:, h * F2:(h + 1) * F2],
                                xl_tiles[s][:, t * 128:(t + 1) * 128],
                                a_tiles[s][:, h * F2:(h + 1) * F2],
                                start=(s == 0), stop=(s == SK - 1),
                            )
                h1l = [h1_pool.tile([128, M_LOC], HALF, name=f"h1l{t}", tag=f"h1l{t}") for t in range(NB)]
                for t in range(NB):
                    nc.scalar.activation(h1l[t][:], psum_L[t][:], AF.Relu, bias=bias_s[:, t:t + 1])
            xl_ctx.__exit__(None, None, None)

            # Y_L, bounce + AllGather-L
            with tc.tile_pool(name="ps_y", bufs=2, space="PSUM") as psy_pool:
                yst = y_pool.tile([128, MB, D_O], HALF, name="ylst", tag="yst")
                for mb in range(MB):
                    psy = psy_pool.tile([128, D_O], F32, name=f"psyl{mb}", tag="psy")
                    for t in range(NB):
                        nc.tensor.matmul(
                            psy[:], h1l[t][:, mb * 128:(mb + 1) * 128], w2_s[:, 0, t, :],
                            start=(t == 0), stop=(t == NB - 1),
                        )
                    nc.vector.tensor_copy(yst[:, mb, :], psy[:])
                for mb in range(MB):
                    nc.sync.dma_start(yl_bounce[mb * 128:(mb + 1) * 128, :], yst[:, mb, :])
            h1_ctx.__exit__(None, None, None)
            if collectives:
                nc.gpsimd.collective_compute(
                    "AllGather", mybir.AluOpType.bypass,
                    ins=[yl_bounce.opt()], outs=[yl_all.opt()],
                    replica_groups=[list(range(CORES))],
                )
            else:
                nc.sync.dma_start(yl_all[0:M_LOC, :], yl_bounce[:])

            # ===== prop2: G leads (its PPMI stream executes while gather-L
            # flies), then 1:1 interleave, L-only compute tail =====
            YQ = 8        # s-chunks per gathered-Y group load
            NQ = SK // YQ
            with (
                tc.tile_pool(name="epi", bufs=1) as e_pool,
            ):
                ys_ctx = tc.tile_pool(name="ys", bufs=8)
                ys_pool = ys_ctx.__enter__()
                ylq, ygq = {}, {}

                def load_y(which, q):
                    src = yl_all if which == "l" else yg_all
                    t = ys_pool.tile([128, YQ, D_O], HALF, name=f"y{which}{q}", tag="ys")
                    nc.sync.dma_start(
                        t[:],
                        src[q * YQ * 128:(q + 1) * YQ * 128, :].rearrange(
                            "(c p) o -> p c o", p=128),
                    )
                    (ylq if which == "l" else ygq)[q] = t

                with tc.tile_pool(name="ps_2", bufs=1, space="PSUM") as ps2_pool:
                    ps_HL = ps2_pool.tile([128, M_LOC], F32, name="ps_HL")
                    ps_HG = ps2_pool.tile([128, M_LOC], F32, name="ps_HG")

                    def l2_chunk(s):
                        q, r = divmod(s, YQ)
                        if r == 0:
                            for dq in (2, 3):
                                if q + dq < NQ and (q + dq) not in ylq:
                                    load_y("l", q + dq)
                        for h in range(NH):
                            nc.tensor.matmul(
                                ps_HL[:, h * F2:(h + 1) * F2],
                                ylq[q][:, r, :],
                                a_tiles[s][:, h * F2:(h + 1) * F2],
                                start=(s == 0), stop=(s == SK - 1),
                            )

                    def g2_chunk(s):
                        q, r = divmod(s, YQ)
                        if r == 0:
                            for dq in (2, 3):
                                if q + dq < NQ and (q + dq) not in ygq:
                                    load_y("g", q + dq)
                        p_s = pg_tiles.pop(s)
                        for h in range(NH):
                            nc.tensor.matmul(
                                ps_HG[:, h * F2:(h + 1) * F2],
                                ygq[q][:, r, :],
                                p_s[:, h * F2:(h + 1) * F2],
                                start=(s == 0), stop=(s == SK - 1),
                            )

                    HEAD = 24  # G-only chunks while gather-L completes
                    load_y("g", 0)
                    load_y("g", 1)
                    load_y("l", 0)
                    load_y("l", 1)
                    for j in range(6):
                        load_p(j)
                    for s in range(HEAD):
                        if s + 6 < SK:
                            load_p(s + 6)
                        g2_chunk(s)
                    ls = 0
                    for s in range(HEAD, SK):
                        if s + 6 < SK:
                            load_p(s + 6)
                        g2_chunk(s)
                        l2_chunk(ls)
                        ls += 1
                    while ls < SK:
                        l2_chunk(ls)
                        ls += 1

                    hlt = e_pool.tile([128, M_LOC], HALF, name="hlt")
                    hgt = e_pool.tile([128, M_LOC], HALF, name="hgt")
                    for h in range(NH):
                        sl = slice(h * F2, (h + 1) * F2)
                        nc.scalar.activation(hlt[:, sl], ps_HL[:, sl], AF.Relu, bias=bias_s[:, 4:5])
                        nc.scalar.activation(hgt[:, sl], ps_HG[:, sl], AF.Relu, bias=bias_s[:, 5:6], scale=1.0 / N)
                ys_ctx.__exit__(None, None, None)

                # ---- attention fusion + classifier, pipelined per m-half ----
                with tc.tile_pool(name="ps_3", bufs=1, space="PSUM") as ps3_pool:
                    ps_sd = ps3_pool.tile([1, M_LOC], F32, name="ps_sd")
                    ps_a0 = ps3_pool.tile([128, M_LOC], F32, name="ps_a0")
                    ps_out = ps3_pool.tile([N_CLS, M_LOC], F32, name="ps_out")
                    a0t = e_pool.tile([1, M_LOC], HALF, name="a0t")
                    d_sb = e_pool.tile([128, M_LOC], HALF, name="d_sb")
                    zt = e_pool.tile([128, M_LOC], HALF, name="zt")
                    out_sb = e_pool.tile([N_CLS, M_LOC], F32, name="out_sb")
                    for h in range(NH):
                        sl = slice(h * F2, (h + 1) * F2)
                        nc.tensor.matmul(ps_sd[:, sl], wadc_s[:, 0:1], hlt[:, sl], start=True, stop=False)
                        nc.tensor.matmul(ps_sd[:, sl], wadc_s[:, 1:2], hgt[:, sl], start=False, stop=True)
                        nc.scalar.activation(a0t[:, sl], ps_sd[:, sl], AF.Sigmoid)
                        nc.tensor.matmul(ps_a0[:, sl], ones_s[:], a0t[:, sl], start=True, stop=True)
                        nc.vector.tensor_sub(d_sb[:, sl], hlt[:, sl], hgt[:, sl])
                        nc.vector.tensor_mul(zt[:, sl], d_sb[:, sl], ps_a0[:, sl])
                        nc.vector.tensor_add(zt[:, sl], zt[:, sl], hgt[:, sl])
                        nc.tensor.matmul(ps_out[:, sl], wadc_s[:, 2:10], zt[:, sl], start=True, stop=True)
                        nc.vector.tensor_scalar_add(out_sb[:, sl], ps_out[:, sl], bias_s[0:N_CLS, 6:7])
                        nc.sync.dma_start(out_d[:, sl], out_sb[:, sl])

    nc.compile()
    return nc


def _prep(inputs):
    """Host-side preprocessing: fold tao into weights, build normalized dense
    adjacency from the edge list, pre-transpose / shard / cast operands."""
    f32 = np.float32
    bf = np.float16
    feats = np.asarray(inputs["feats"], f32)
    norm = np.asarray(inputs["norm"], f32)
    PPMI = np.asarray(inputs["PPMI"], f32)
    src = np.asarray(inputs["src"]).astype(np.int64)
    dst = np.asarray(inputs["dst"]).astype(np.int64)

    w1L = np.asarray(inputs["w1"], f32) @ np.asarray(inputs["tao_1_L"], f32)
    w1G = np.asarray(inputs["w1g"], f32) @ np.asarray(inputs["tao_1_G"], f32)
    w2L = np.asarray(inputs["w2"], f32) @ np.asarray(inputs["tao_2_L"], f32)
    w2G = np.asarray(inputs["w2g"], f32) @ np.asarray(inputs["tao_2_G"], f32)
    W_a = np.asarray(inputs["W_a"], f32)
    W_c = np.asarray(inputs["W_c"], f32)

    # dense normalized adjacency, pre-transposed: AT[s, d] = norm[d]*norm[s]*count(s->d)
    nv = norm[:, 0]
    AT = np.zeros((N, N), f32)
    np.add.at(AT, (src, dst), nv[src] * nv[dst])
    AT_bf = AT.astype(bf)
    PT_bf = (np.ascontiguousarray(PPMI.T) * np.float32(N)).astype(bf)

    wad = (W_a[:, 0] - W_a[:, 1]).astype(f32)  # [256]

    biases = np.zeros((128, 7), f32)
    biases[:, 0:2] = np.asarray(inputs["b1"], f32).reshape(NB, 128).T
    biases[:, 2:4] = np.asarray(inputs["b1g"], f32).reshape(NB, 128).T
    biases[:, 4] = np.asarray(inputs["b2"], f32)
    biases[:, 5] = np.asarray(inputs["b2g"], f32)
    biases[:N_CLS, 6] = np.asarray(inputs["b_c"], f32)
    wadc = np.zeros((128, 10), f32)
    wadc[:, 0] = wad[:128]
    wadc[:, 1] = wad[128:]
    wadc[:, 2:10] = W_c

    common = {
        "ftT": np.ascontiguousarray(feats.T).astype(bf),
        "w_both": np.concatenate([w1L, w1G], axis=1).astype(bf),
        "w2lg": np.stack([w2L, w2G]).astype(bf),
        "biases": biases,
        "wadc": wadc.astype(bf),
    }
    in_maps = []
    for c in range(CORES):
        sel = slice(c * M_LOC, (c + 1) * M_LOC)
        m = dict(common)
        m["a_t"] = np.ascontiguousarray(AT_bf[:, sel])
        m["p_t"] = np.ascontiguousarray(PT_bf[:, sel])
        in_maps.append(m)
    return in_maps


def kernel(**inputs) -> np.ndarray:
    if "nc" not in _CACHE:
        _CACHE["nc"] = _build()
    nc = _CACHE["nc"]
    in_maps = _prep(inputs)
    res = run_bass_kernel_spmd(nc, in_maps, list(range(CORES)), trace=False)
    out = np.empty((N, N_CLS), np.float32)
    for c in range(CORES):
        out[c * M_LOC:(c + 1) * M_LOC, :] = res.results[c]["outT"].T
    return out


if __name__ == "__main__":
    rng = np.random.default_rng(0)
    dummy = {
        "feats": rng.standard_normal((N, D_IN)).astype(np.float32),
        "norm": rng.random((N, 1)).astype(np.float32),
        "tao_1_L": rng.standard_normal((D_H, D_H)).astype(np.float32) / 16,
        "tao_2_L": rng.standard_normal((D_O, D_O)).astype(np.float32) / 11,
        "tao_1_G": rng.standard_normal((D_H, D_H)).astype(np.float32) / 16,
        "tao_2_G": rng.standard_normal((D_O, D_O)).astype(np.float32) / 11,
        "PPMI": rng.random((N, N)).astype(np.float32) / N,
        "w1": rng.random((D_IN, D_H)).astype(np.float32) * 0.06,
        "b1": rng.random((D_H,)).astype(np.float32) * 0.04,
        "w2": rng.random((D_H, D_O)).astype(np.float32) * 0.09,
        "b2": rng.random((D_O,)).astype(np.float32) * 0.06,
        "w1g": rng.random((D_IN, D_H)).astype(np.float32) * 0.06,
        "b1g": rng.random((D_H,)).astype(np.float32) * 0.04,
        "w2g": rng.random((D_H, D_O)).astype(np.float32) * 0.09,
        "b2g": rng.random((D_O,)).astype(np.float32) * 0.06,
        "W_a": rng.random((2 * D_O, 2)).astype(np.float32) * 0.7,
        "W_c": rng.random((D_O, N_CLS)).astype(np.float32) * 0.35,
        "b_c": rng.random((N_CLS,)).astype(np.float32) * 0.35,
        "src": rng.integers(0, N, (262144,)).astype(np.int32),
        "dst": rng.integers(0, N, (262144,)).astype(np.int32),
    }
    out = kernel(**dummy)
    print("out", out.shape, out.dtype, np.abs(out).mean())



# revision 5
# speedup vs baseline: 1.2150x; 1.2150x over previous
"""Trainium2 Bass kernel for nn_MetaLearner (dual-branch GCN + PPMI meta-learner).

Strategy (v2: fp8 DoubleRow propagation)
----------------------------------------
Host folds the edge-list GCN into dense matrices and factors out everything
that fp8 cannot represent exactly:

  local branch:  A' = diag(n) . C . diag(n), C[s,m] = #edges(s->m) is a small
                 INTEGER matrix -> exact in fp8e4. The diag(n) row scale is
                 folded into the X/Y operands (per-partition ACT scale), the
                 diag(n) column scale is applied with a DVE broadcast multiply.
  global branch: P = N*PPMI^T is uniform[0,1); mean-shift P = 0.5 + Q with
                 Q in fp8e4 and the exact rank-1 correction 0.5*colsum(X)
                 folded into the activation bias (host-computed for prop1,
                 device-computed + all-gathered for prop2).

All four N x N propagation matmuls then run as fp8 DoubleRow pairs (2 k-tiles
per instruction at 0.5 cycles/row = 4x fp16) with hi+lo fp8 splits of the
X/Y operands (quantization residual is itself fp8-encoded, so the pair of
DoubleRow passes is ~2x faster than fp16 at fp16-level accuracy).

Distribution: output rows sharded 1024/core. X = feats @ [W1L|W1G] is
computed on each core for its own row slab only, split hi/lo to fp8 and
AllGathered (8x less PE than the replicated-X baseline). C and Q column
slabs (8 MB each in fp8) stay RESIDENT in SBUF and are loaded once --
the baseline streamed PPMI twice at fp16.

Per-core timeline: X (fp16, 8 chunks) -> gather-X || load C/Q ->
prop1 L-hi, G-hi, then a shared lo pass (X-lo streamed once for both
branches) -> Y both branches + fp8 split + colsum(Y_G) -> gather-Y ->
prop2 (everything resident) -> fused attention + classifier.
"""

import sys

sys.path.insert(0, "/opt/trn_rl_repo")

import numpy as np
import ml_dtypes

import concourse.bacc as bacc
import concourse.mybir as mybir
import concourse.tile as tile
from concourse.bass_utils import run_bass_kernel_spmd

N = 8192
D_IN = 512
D_H = 256
D_O = 128
N_CLS = 8
CORES = 8
M_LOC = N // CORES          # 1024 rows per core
NPAIR = N // 256            # 32 DoubleRow k-tile pairs
KC = D_IN // 128            # 4 k-chunks of input features
NB = D_H // 128             # 2 n-blocks of hidden features
F2 = 512                    # matmul free-dim slice
NH = M_LOC // F2            # 2 free-dim halves of the local rows
MB = M_LOC // 128           # 8 local row blocks
SC = 8                      # local s-chunks (X compute)
YSCALE = 256.0              # pre-scale for Y_G so fp8 stays in normal range

E4 = ml_dtypes.float8_e4m3
HALF = mybir.dt.float16
F8 = mybir.dt.float8e4
F32 = mybir.dt.float32
AF = mybir.ActivationFunctionType
ALU = mybir.AluOpType
DR = mybir.MatmulPerfMode.DoubleRow

_CACHE = {}


def _build(collectives: bool = True):
    nc = bacc.Bacc("TRN2", target_bir_lowering=False, debug=False, num_devices=CORES)

    ftT_d = nc.dram_tensor("ftT", [D_IN, M_LOC], HALF, kind="ExternalInput")
    wb_d = nc.dram_tensor("w_both", [D_IN, 2 * D_H], HALF, kind="ExternalInput")
    w2lg_d = nc.dram_tensor("w2lg", [2, D_H, D_O], HALF, kind="ExternalInput")
    c8_d = nc.dram_tensor("c8", [N, M_LOC], F8, kind="ExternalInput")
    q8_d = nc.dram_tensor("q8", [N, M_LOC], F8, kind="ExternalInput")
    # biases packed [128, 8] f32: 0-1 b1, 2-3 b1g_eff, 4 b2, 5 b2g, 6 b_c (rows 0-7)
    bias_d = nc.dram_tensor("biases", [128, 8], F32, kind="ExternalInput")
    # wadc packed [128, 10] fp16: 0 wad_L, 1 wad_G, 2-9 W_c
    wadc_d = nc.dram_tensor("wadc", [128, 10], HALF, kind="ExternalInput")
    nvec_d = nc.dram_tensor("nvec", [128, SC], F32, kind="ExternalInput")
    nrow_d = nc.dram_tensor("nrow", [1, M_LOC], HALF, kind="ExternalInput")
    out_d = nc.dram_tensor("outT", [N_CLS, M_LOC], F32, kind="ExternalOutput")

    ftT_v = ftT_d[:].rearrange("(kc p) s -> p kc s", p=128)
    c8_v = c8_d[:].rearrange("(q pair p) m -> p q pair m", p=128, pair=2)
    q8_v = q8_d[:].rearrange("(q pair p) m -> p q pair m", p=128, pair=2)

    with tile.TileContext(nc) as tc:
        with (
            tc.tile_pool(name="const", bufs=1) as cpool,
            tc.tile_pool(name="cq", bufs=1) as cq_pool,
            tc.tile_pool(name="dram", bufs=1, space="DRAM") as dram,
        ):
            # ---- constants (gpsimd DMA queue) ----
            bias_s = cpool.tile([128, 8], F32, name="bias_s")
            nc.gpsimd.dma_start(bias_s[:], bias_d[:])
            wadc_s = cpool.tile([128, 10], HALF, name="wadc_s")
            nc.gpsimd.dma_start(wadc_s[:], wadc_d[:])
            nvec_s = cpool.tile([128, SC], F32, name="nvec_s")
            nc.gpsimd.dma_start(nvec_s[:], nvec_d[:])
            nrow_s = cpool.tile([1, M_LOC], HALF, name="nrow_s")
            nc.gpsimd.dma_start(nrow_s[:], nrow_d[:])
            w2_s = cpool.tile([128, 2, NB, D_O], HALF, name="w2_s")
            nc.gpsimd.dma_start(w2_s[:], w2lg_d[:].rearrange("b (c p) m -> p b c m", p=128))
            ones_s = cpool.tile([1, 128], HALF, name="ones_s")
            nc.gpsimd.memset(ones_s[:], 1.0)
            ones128_8 = cpool.tile([128, 1], F8, name="ones128_8")
            nc.gpsimd.memset(ones128_8[:], 1.0)
            ones8_f = cpool.tile([8, 1], F32, name="ones8_f")
            nc.gpsimd.memset(ones8_f[:], 1.0)
            # warm the sigmoid table set (relu/copy are in every set)
            sig_warm = cpool.tile([1, 8], HALF, name="sig_warm")
            nc.scalar.activation(sig_warm[:], ones_s[:1, 0:8], AF.Sigmoid)

            # ---- sync queue: weights/features then the resident C/Q slabs ----
            wb_s = cpool.tile([128, KC, 2 * D_H], HALF, name="wb_s")
            nc.sync.dma_start(wb_s[:], wb_d[:].rearrange("(kc p) m -> p kc m", p=128))
            ft_ctx = tc.tile_pool(name="ft", bufs=1)
            ft_pool = ft_ctx.__enter__()
            ft_s = ft_pool.tile([128, KC, M_LOC], HALF, name="ft_s")
            nc.sync.dma_start(ft_s[:], ftT_v)
            c_all = cq_pool.tile([128, NPAIR, 2, M_LOC], F8, name="c_all")
            q_all = cq_pool.tile([128, NPAIR, 2, M_LOC], F8, name="q_all")
            GQ = 8  # pair-tiles per group DMA
            NG = NPAIR // GQ
            for g in range(NG):
                nc.sync.dma_start(c_all[:, g * GQ:(g + 1) * GQ], c8_v[:, g * GQ:(g + 1) * GQ])
                nc.sync.dma_start(q_all[:, g * GQ:(g + 1) * GQ], q8_v[:, g * GQ:(g + 1) * GQ])

            # bounce + gathered tensors
            xb_dram = dram.tile([M_LOC, 2 * (2 * D_H)], F8, name="xb_dram")
            x_all_dram = dram.tile([N, 2 * (2 * D_H)], F8, addr_space="Shared", name="x_all")
            yb_dram = dram.tile([M_LOC, 4 * D_O], F8, name="yb_dram")
            y_all_dram = dram.tile([N, 4 * D_O], F8, addr_space="Shared", name="y_all")
            csb_dram = dram.tile([1, D_O], F32, name="csb_dram")
            cs_all_dram = dram.tile([CORES, D_O], F32, addr_space="Shared", name="cs_all")
            x_all_v = x_all_dram[:].rearrange("(q pair p) c -> p q pair c", p=128, pair=2)
            y_all_v = y_all_dram[:].rearrange("(q pair p) c -> p q pair c", p=128, pair=2)

            # ===== X = feats_slab @ [W1L | W1G] (fp16), split to fp8 hi/lo =====
            # xb columns: [hi_L 256 | hi_G 256 | lo_L 256 | lo_G 256]
            nbc_s = cpool.tile([128, M_LOC], HALF, name="nbc_s")
            with (
                tc.tile_pool(name="xstage", bufs=2) as xs_pool,
                tc.tile_pool(name="ps_x", bufs=2, space="PSUM") as psx_pool,
                tc.tile_pool(name="ps_nbc", bufs=1, space="PSUM") as psn_pool,
            ):
                # n broadcast tile for the free-dim diag(n) of the L branch
                ps_nbc = psn_pool.tile([128, M_LOC], F32, name="ps_nbc")
                for h in range(NH):
                    nc.tensor.matmul(ps_nbc[:, h * F2:(h + 1) * F2], ones_s[:],
                                     nrow_s[:, h * F2:(h + 1) * F2], start=True, stop=True)
                nc.scalar.activation(nbc_s[:], ps_nbc[:], AF.Copy)

                for i in range(SC):
                    psx = psx_pool.tile([128, 2 * D_H], F32, name=f"psx{i}", tag="psx")
                    for k in range(KC):
                        nc.tensor.matmul(
                            psx[:], ft_s[:, k, i * 128:(i + 1) * 128], wb_s[:, k, :],
                            start=(k == 0), stop=(k == KC - 1),
                        )
                    xst = xs_pool.tile([128, 2 * (2 * D_H)], F8, name=f"xst{i}", tag="xst")
                    nsc = nvec_s[:, i:i + 1]
                    # hi_L = fp8(n * x_L); lo_L = fp8(n * x_L - hi_L)
                    nc.scalar.activation(xst[:, 0:D_H], psx[:, 0:D_H], AF.Copy, scale=nsc)
                    nc.scalar.activation(xst[:, D_H:2 * D_H], psx[:, D_H:2 * D_H], AF.Copy)
                    nc.vector.scalar_tensor_tensor(
                        xst[:, 2 * D_H:3 * D_H], psx[:, 0:D_H], nsc, xst[:, 0:D_H],
                        op0=ALU.mult, op1=ALU.subtract)
                    nc.vector.scalar_tensor_tensor(
                        xst[:, 3 * D_H:4 * D_H], psx[:, D_H:2 * D_H], 1.0, xst[:, D_H:2 * D_H],
                        op0=ALU.mult, op1=ALU.subtract)
                    nc.scalar.dma_start(xb_dram[i * 128:(i + 1) * 128, :], xst[:])

            if collectives:
                nc.gpsimd.collective_compute(
                    "AllGather", ALU.bypass,
                    ins=[xb_dram.opt()], outs=[x_all_dram.opt()],
                    replica_groups=[list(range(CORES))],
                )
            else:
                nc.gpsimd.dma_start(x_all_dram[0:M_LOC, :], xb_dram[:])
            ft_ctx.__exit__(None, None, None)

            # ===== prop1: H1 = act(prop(X)) for both branches =====
            h1_ctx = tc.tile_pool(name="h1", bufs=1)
            h1_pool = h1_ctx.__enter__()
            xh_ctx = tc.tile_pool(name="xh", bufs=1)
            xh_pool = xh_ctx.__enter__()
            xh_all = xh_pool.tile([128, NPAIR, 2, 2 * D_H], F8, name="xh_all")
            for g in range(NG):
                nc.scalar.dma_start(xh_all[:, g * GQ:(g + 1) * GQ],
                                    x_all_v[:, g * GQ:(g + 1) * GQ, :, 0:2 * D_H])

            with tc.tile_pool(name="ps_1", bufs=1, space="PSUM") as ps1_pool:
                psum_L = [ps1_pool.tile([128, M_LOC], F32, name=f"psl{t}") for t in range(NB)]
                psum_G = [ps1_pool.tile([128, M_LOC], F32, name=f"psg{t}") for t in range(NB)]

                # hi passes: L then G (fed by the c/q resident loads)
                for q in range(NPAIR):
                    for t in range(NB):
                        for h in range(NH):
                            nc.tensor.matmul(
                                psum_L[t][:, h * F2:(h + 1) * F2],
                                xh_all[:, q, :, t * 128:(t + 1) * 128],
                                c_all[:, q, :, h * F2:(h + 1) * F2],
                                start=(q == 0), stop=False, perf_mode=DR,
                            )
                for q in range(NPAIR):
                    for t in range(NB):
                        for h in range(NH):
                            nc.tensor.matmul(
                                psum_G[t][:, h * F2:(h + 1) * F2],
                                xh_all[:, q, :, D_H + t * 128:D_H + (t + 1) * 128],
                                q_all[:, q, :, h * F2:(h + 1) * F2],
                                start=(q == 0), stop=False, perf_mode=DR,
                            )
                # shared lo pass: stream X-lo once, use for both branches
                with tc.tile_pool(name="xl", bufs=6) as xl_pool:
                    xl_tiles = {}

                    def load_xl(q):
                        t8 = xl_pool.tile([128, 2, 2 * D_H], F8, name=f"xl{q}", tag="xl")
                        nc.scalar.dma_start(t8[:], x_all_v[:, q, :, 2 * D_H:4 * D_H])
                        xl_tiles[q] = t8

                    for j in range(4):
                        load_xl(j)
                    for q in range(NPAIR):
                        if q + 4 < NPAIR:
                            load_xl(q + 4)
                        xlt = xl_tiles.pop(q)
                        last = (q == NPAIR - 1)
                        for t in range(NB):
                            for h in range(NH):
                                nc.tensor.matmul(
                                    psum_L[t][:, h * F2:(h + 1) * F2],
                                    xlt[:, :, t * 128:(t + 1) * 128],
                                    c_all[:, q, :, h * F2:(h + 1) * F2],
                                    start=False, stop=last, perf_mode=DR,
                                )
                        for t in range(NB):
                            for h in range(NH):
                                nc.tensor.matmul(
                                    psum_G[t][:, h * F2:(h + 1) * F2],
                                    xlt[:, :, D_H + t * 128:D_H + (t + 1) * 128],
                                    q_all[:, q, :, h * F2:(h + 1) * F2],
                                    start=False, stop=last, perf_mode=DR,
                                )

                # activations: H1_L = relu(n_m * agg + b1), H1_G = relu(psum/N + b1g_eff)
                h1l = [h1_pool.tile([128, M_LOC], HALF, name=f"h1l{t}") for t in range(NB)]
                h1g = [h1_pool.tile([128, M_LOC], HALF, name=f"h1g{t}") for t in range(NB)]
                tmp_ctx = tc.tile_pool(name="tmp1", bufs=2)
                tmp_pool = tmp_ctx.__enter__()
                for t in range(NB):
                    tmp = tmp_pool.tile([128, M_LOC], HALF, name=f"tm{t}", tag="tm")
                    nc.vector.tensor_mul(tmp[:], psum_L[t][:], nbc_s[:])
                    nc.scalar.activation(h1l[t][:], tmp[:], AF.Relu, bias=bias_s[:, t:t + 1])
                    nc.scalar.activation(h1g[t][:], psum_G[t][:], AF.Relu,
                                         bias=bias_s[:, 2 + t:3 + t], scale=1.0 / N)
                tmp_ctx.__exit__(None, None, None)
            xh_ctx.__exit__(None, None, None)

            # ===== Y = H1 @ W2 (both branches), fp8 hi/lo split, colsum(Y_G) =====
            # yb columns: [hi_L 128 | lo_L 128 | hi_G 128 | lo_G 128]
            with (
                tc.tile_pool(name="ystage", bufs=2) as ys_pool,
                tc.tile_pool(name="ps_y", bufs=4, space="PSUM") as psy_pool,
                tc.tile_pool(name="ps_cs", bufs=1, space="PSUM") as pcs_pool,
            ):
                ps_cs = pcs_pool.tile([1, D_O], F32, name="ps_cs")
                for mb in range(MB):
                    yst = ys_pool.tile([128, 4 * D_O], F8, name=f"yst{mb}", tag="yst")
                    psyl = psy_pool.tile([128, D_O], F32, name=f"pyl{mb}", tag="psy")
                    psyg = psy_pool.tile([128, D_O], F32, name=f"pyg{mb}", tag="psy")
                    for t in range(NB):
                        nc.tensor.matmul(psyl[:], h1l[t][:, mb * 128:(mb + 1) * 128],
                                         w2_s[:, 0, t, :], start=(t == 0), stop=(t == NB - 1))
                    for t in range(NB):
                        nc.tensor.matmul(psyg[:], h1g[t][:, mb * 128:(mb + 1) * 128],
                                         w2_s[:, 1, t, :], start=(t == 0), stop=(t == NB - 1))
                    nsc = nvec_s[:, mb:mb + 1]
                    nc.scalar.activation(yst[:, 0:D_O], psyl[:], AF.Copy, scale=nsc)
                    nc.vector.scalar_tensor_tensor(
                        yst[:, D_O:2 * D_O], psyl[:], nsc, yst[:, 0:D_O],
                        op0=ALU.mult, op1=ALU.subtract)
                    nc.scalar.activation(yst[:, 2 * D_O:3 * D_O], psyg[:], AF.Copy)
                    nc.vector.scalar_tensor_tensor(
                        yst[:, 3 * D_O:4 * D_O], psyg[:], 1.0, yst[:, 2 * D_O:3 * D_O],
                        op0=ALU.mult, op1=ALU.subtract)
                    # local partial colsum of Y_G' (hi + lo)
                    nc.tensor.matmul(ps_cs[:], ones128_8[:], yst[:, 2 * D_O:3 * D_O],
                                     start=(mb == 0), stop=False)
                    nc.tensor.matmul(ps_cs[:], ones128_8[:], yst[:, 3 * D_O:4 * D_O],
                                     start=False, stop=(mb == MB - 1))
                    nc.scalar.dma_start(yb_dram[mb * 128:(mb + 1) * 128, :], yst[:])
                cs_sb = cpool.tile([1, D_O], F32, name="cs_sb")
                nc.scalar.activation(cs_sb[:], ps_cs[:], AF.Copy)
                nc.scalar.dma_start(csb_dram[:], cs_sb[:])
            h1_ctx.__exit__(None, None, None)

            if collectives:
                nc.gpsimd.collective_compute(
                    "AllGather", ALU.bypass,
                    ins=[yb_dram.opt()], outs=[y_all_dram.opt()],
                    replica_groups=[list(range(CORES))],
                )
                nc.gpsimd.collective_compute(
                    "AllGather", ALU.bypass,
                    ins=[csb_dram.opt()], outs=[cs_all_dram.opt()],
                    replica_groups=[list(range(CORES))],
                )
            else:
                nc.gpsimd.dma_start(y_all_dram[0:M_LOC, :], yb_dram[:])
                nc.gpsimd.dma_start(cs_all_dram[0:1, :], csb_dram[:])

            # ===== prop2 (C/Q resident, Y gathered) + fused epilogue =====
            with (
                tc.tile_pool(name="ys2", bufs=1) as ys2_pool,
                tc.tile_pool(name="epi", bufs=1) as e_pool,
            ):
                y_all = ys2_pool.tile([128, NPAIR, 2, 4 * D_O], F8, name="y_all_s")
                for g in range(NG):
                    nc.scalar.dma_start(y_all[:, g * GQ:(g + 1) * GQ],
                                        y_all_v[:, g * GQ:(g + 1) * GQ])
                cs8 = e_pool.tile([CORES, D_O], F32, name="cs8")
                nc.scalar.dma_start(cs8[:], cs_all_dram[:])
                bias_g2 = e_pool.tile([128, 1], F32, name="bias_g2")

                with tc.tile_pool(name="ps_b", bufs=1, space="PSUM") as psb_pool:
                    ps_b = psb_pool.tile([128, 1], F32, name="ps_b")
                    nc.tensor.matmul(ps_b[:], cs8[:], ones8_f[:], start=True, stop=True)
                    nc.vector.scalar_tensor_tensor(
                        bias_g2[:], ps_b[:], 0.5 / (YSCALE * N), bias_s[:, 5:6],
                        op0=ALU.mult, op1=ALU.add)

                hlt = e_pool.tile([128, M_LOC], HALF, name="hlt")
                hgt = e_pool.tile([128, M_LOC], HALF, name="hgt")
                with tc.tile_pool(name="ps_2", bufs=1, space="PSUM") as ps2_pool:
                    ps_HL = ps2_pool.tile([128, M_LOC], F32, name="ps_HL")
                    ps_HG = ps2_pool.tile([128, M_LOC], F32, name="ps_HG")
                    for q in range(NPAIR):
                        first, last = (q == 0), (q == NPAIR - 1)
                        for sub in range(2):
                            for h in range(NH):
                                nc.tensor.matmul(
                                    ps_HL[:, h * F2:(h + 1) * F2],
                                    y_all[:, q, :, sub * D_O:(sub + 1) * D_O],
                                    c_all[:, q, :, h * F2:(h + 1) * F2],
                                    start=(first and sub == 0), stop=(last and sub == 1),
                                    perf_mode=DR,
                                )
                        for sub in range(2):
                            for h in range(NH):
                                nc.tensor.matmul(
                                    ps_HG[:, h * F2:(h + 1) * F2],
                                    y_all[:, q, :, (2 + sub) * D_O:(3 + sub) * D_O],
                                    q_all[:, q, :, h * F2:(h + 1) * F2],
                                    start=(first and sub == 0), stop=(last and sub == 1),
                                    perf_mode=DR,
                                )
                    tmp2_ctx = tc.tile_pool(name="tmp2", bufs=1)
                    tmp2_pool = tmp2_ctx.__enter__()
                    tmp2 = tmp2_pool.tile([128, M_LOC], HALF, name="tmp2")
                    nc.vector.tensor_mul(tmp2[:], ps_HL[:], nbc_s[:])
                    for h in range(NH):
                        sl = slice(h * F2, (h + 1) * F2)
                        nc.scalar.activation(hlt[:, sl], tmp2[:, sl], AF.Relu,
                                             bias=bias_s[:, 4:5])
                        nc.scalar.activation(hgt[:, sl], ps_HG[:, sl], AF.Relu,
                                             bias=bias_g2[:, 0:1], scale=1.0 / (YSCALE * N))
                    tmp2_ctx.__exit__(None, None, None)

                # ---- attention fusion + classifier, pipelined per m-half ----
                with tc.tile_pool(name="ps_3", bufs=1, space="PSUM") as ps3_pool:
                    ps_sd = ps3_pool.tile([1, M_LOC], F32, name="ps_sd")
                    ps_a0 = ps3_pool.tile([128, M_LOC], F32, name="ps_a0")
                    ps_out = ps3_pool.tile([N_CLS, M_LOC], F32, name="ps_out")
                    a0t = e_pool.tile([1, M_LOC], HALF, name="a0t")
                    d_sb = e_pool.tile([128, M_LOC], HALF, name="d_sb")
                    zt = e_pool.tile([128, M_LOC], HALF, name="zt")
                    out_sb = e_pool.tile([N_CLS, M_LOC], F32, name="out_sb")
                    for h in range(NH):
                        sl = slice(h * F2, (h + 1) * F2)
                        nc.tensor.matmul(ps_sd[:, sl], wadc_s[:, 0:1], hlt[:, sl], start=True, stop=False)
                        nc.tensor.matmul(ps_sd[:, sl], wadc_s[:, 1:2], hgt[:, sl], start=False, stop=True)
                        nc.scalar.activation(a0t[:, sl], ps_sd[:, sl], AF.Sigmoid)
                        nc.tensor.matmul(ps_a0[:, sl], ones_s[:], a0t[:, sl], start=True, stop=True)
                        nc.vector.tensor_sub(d_sb[:, sl], hlt[:, sl], hgt[:, sl])
                        nc.vector.tensor_mul(zt[:, sl], d_sb[:, sl], ps_a0[:, sl])
                        nc.vector.tensor_add(zt[:, sl], zt[:, sl], hgt[:, sl])
                        nc.tensor.matmul(ps_out[:, sl], wadc_s[:, 2:10], zt[:, sl], start=True, stop=True)
                        nc.vector.tensor_scalar_add(out_sb[:, sl], ps_out[:, sl], bias_s[0:N_CLS, 6:7])
                        nc.scalar.dma_start(out_d[:, sl], out_sb[:, sl])

    nc.compile()
    return nc


def _prep(inputs):
    """Host-side preprocessing: fold tao into weights, build the integer edge
    count matrix and the mean-shifted PPMI slab, shard / cast operands."""
    f32 = np.float32
    bf = np.float16
    feats = np.asarray(inputs["feats"], f32)
    norm = np.asarray(inputs["norm"], f32)
    PPMI = np.asarray(inputs["PPMI"], f32)
    src = np.asarray(inputs["src"]).astype(np.int64)
    dst = np.asarray(inputs["dst"]).astype(np.int64)

    w1L = np.asarray(inputs["w1"], f32) @ np.asarray(inputs["tao_1_L"], f32)
    w1G = np.asarray(inputs["w1g"], f32) @ np.asarray(inputs["tao_1_G"], f32)
    w2L = np.asarray(inputs["w2"], f32) @ np.asarray(inputs["tao_2_L"], f32)
    w2G = np.asarray(inputs["w2g"], f32) @ np.asarray(inputs["tao_2_G"], f32)
    W_a = np.asarray(inputs["W_a"], f32)
    W_c = np.asarray(inputs["W_c"], f32)

    nv = norm[:, 0]
    # integer edge-count matrix C[s, m] = #edges(s->m): exact in fp8e4
    C = np.zeros((N, N), f32)
    np.add.at(C, (src, dst), 1.0)
    C8 = C.astype(E4)
    # mean-shifted PPMI^T: Q = N*PPMI^T - 0.5 in fp8e4
    Q8 = (np.ascontiguousarray(PPMI.T) * np.float32(N) - np.float32(0.5)).astype(E4)

    # rank-1 mean correction for prop1-G, folded into the bias (host-exact)
    colsum_XG = (feats.sum(axis=0) @ w1G).astype(f32)
    b1g_eff = np.asarray(inputs["b1g"], f32) + np.float32(0.5 / N) * colsum_XG

    wad = (W_a[:, 0] - W_a[:, 1]).astype(f32)

    biases = np.zeros((128, 8), f32)
    biases[:, 0:2] = np.asarray(inputs["b1"], f32).reshape(NB, 128).T
    biases[:, 2:4] = b1g_eff.reshape(NB, 128).T
    biases[:, 4] = np.asarray(inputs["b2"], f32)
    biases[:, 5] = np.asarray(inputs["b2g"], f32)
    biases[:N_CLS, 6] = np.asarray(inputs["b_c"], f32)
    wadc = np.zeros((128, 10), f32)
    wadc[:, 0] = wad[:128]
    wadc[:, 1] = wad[128:]
    wadc[:, 2:10] = W_c

    common = {
        "w_both": np.concatenate([w1L, w1G], axis=1).astype(bf),
        "w2lg": np.stack([w2L, np.float32(YSCALE) * w2G]).astype(bf),
        "biases": biases,
        "wadc": wadc.astype(bf),
    }
    in_maps = []
    for c in range(CORES):
        sel = slice(c * M_LOC, (c + 1) * M_LOC)
        m = dict(common)
        m["ftT"] = np.ascontiguousarray(feats[sel].T).astype(bf)
        m["c8"] = np.ascontiguousarray(C8[:, sel])
        m["q8"] = np.ascontiguousarray(Q8[:, sel])
        m["nvec"] = np.ascontiguousarray(nv[sel].reshape(SC, 128).T).astype(f32)
        m["nrow"] = nv[sel][None, :].astype(bf)
        in_maps.append(m)
    return in_maps


def kernel(**inputs) -> np.ndarray:
    if "nc" not in _CACHE:
        _CACHE["nc"] = _build()
    nc = _CACHE["nc"]
    in_maps = _prep(inputs)
    res = run_bass_kernel_spmd(nc, in_maps, list(range(CORES)), trace=False)
    out = np.empty((N, N_CLS), np.float32)
    for c in range(CORES):
        out[c * M_LOC:(c + 1) * M_LOC, :] = res.results[c]["outT"].T
    return out


if __name__ == "__main__":
    rng = np.random.default_rng(0)
    dummy = {
        "feats": rng.standard_normal((N, D_IN)).astype(np.float32),
        "norm": rng.random((N, 1)).astype(np.float32),
        "tao_1_L": rng.standard_normal((D_H, D_H)).astype(np.float32) / 16,
        "tao_2_L": rng.standard_normal((D_O, D_O)).astype(np.float32) / 11,
        "tao_1_G": rng.standard_normal((D_H, D_H)).astype(np.float32) / 16,
        "tao_2_G": rng.standard_normal((D_O, D_O)).astype(np.float32) / 11,
        "PPMI": rng.random((N, N)).astype(np.float32) / N,
        "w1": rng.random((D_IN, D_H)).astype(np.float32) * 0.06,
        "b1": rng.random((D_H,)).astype(np.float32) * 0.04,
        "w2": rng.random((D_H, D_O)).astype(np.float32) * 0.09,
        "b2": rng.random((D_O,)).astype(np.float32) * 0.06,
        "w1g": rng.random((D_IN, D_H)).astype(np.float32) * 0.06,
        "b1g": rng.random((D_H,)).astype(np.float32) * 0.04,
        "w2g": rng.random((D_H, D_O)).astype(np.float32) * 0.09,
        "b2g": rng.random((D_O,)).astype(np.float32) * 0.06,
        "W_a": rng.random((2 * D_O, 2)).astype(np.float32) * 0.7,
        "W_c": rng.random((D_O, N_CLS)).astype(np.float32) * 0.35,
        "b_c": rng.random((N_CLS,)).astype(np.float32) * 0.35,
        "src": rng.integers(0, N, (262144,)).astype(np.int32),
        "dst": rng.integers(0, N, (262144,)).astype(np.int32),
    }
    out = kernel(**dummy)
    print("out", out.shape, out.dtype, np.abs(out).mean())


# revision 42
# speedup vs baseline: 1.5192x; 1.2503x over previous
"""Trainium2 Bass kernel for nn_MetaLearner (dual-branch GCN + PPMI meta-learner).

Strategy (v2: fp8 DoubleRow propagation)
----------------------------------------
Host folds the edge-list GCN into dense matrices and factors out everything
that fp8 cannot represent exactly:

  local branch:  A' = diag(n) . C . diag(n), C[s,m] = #edges(s->m) is a small
                 INTEGER matrix -> exact in fp8e4. The diag(n) row scale is
                 folded into the X/Y operands (per-partition ACT scale), the
                 diag(n) column scale is applied with a DVE broadcast multiply.
  global branch: P = N*PPMI^T is uniform[0,1); mean-shift P = 0.5 + Q with
                 Q in fp8e4 and the exact rank-1 correction 0.5*colsum(X)
                 folded into the activation bias (host-computed for prop1,
                 device-computed + all-gathered for prop2).

All four N x N propagation matmuls then run as fp8 DoubleRow pairs (2 k-tiles
per instruction at 0.5 cycles/row = 4x fp16) with hi+lo fp8 splits of the
X/Y operands (quantization residual is itself fp8-encoded, so the pair of
DoubleRow passes is ~2x faster than fp16 at fp16-level accuracy).

Distribution: output rows sharded 1024/core. X = feats @ [W1L|W1G] is
computed on each core for its own row slab only, split hi/lo to fp8 and
AllGathered (8x less PE than the replicated-X baseline). C and Q column
slabs (8 MB each in fp8) stay RESIDENT in SBUF and are loaded once --
the baseline streamed PPMI twice at fp16.

Per-core timeline: X (fp16, 8 chunks) -> gather-X || load C/Q ->
prop1 L-hi, G-hi, then a shared lo pass (X-lo streamed once for both
branches) -> Y both branches + fp8 split + colsum(Y_G) -> gather-Y ->
prop2 (everything resident) -> fused attention + classifier.
"""

import os
import sys

sys.path.insert(0, "/opt/trn_rl_repo")

import numpy as np
import ml_dtypes

import concourse.bacc as bacc
import concourse.mybir as mybir
import concourse.tile as tile
from concourse.bass_utils import run_bass_kernel_spmd

N = 8192
D_IN = 512
D_H = 256
D_O = 128
N_CLS = 8
CORES = 8
M_LOC = N // CORES          # 1024 rows per core
NPAIR = N // 256            # 32 DoubleRow k-tile pairs
KC = D_IN // 128            # 4 k-chunks of input features
NB = D_H // 128             # 2 n-blocks of hidden features
F2 = 512                    # matmul free-dim slice
NH = M_LOC // F2            # 2 free-dim halves of the local rows
MB = M_LOC // 128           # 8 local row blocks
SC = 8                      # local s-chunks (X compute)
YSCALE = 256.0              # pre-scale for Y_G so fp8 stays in normal range

E4 = ml_dtypes.float8_e4m3
HALF = mybir.dt.float16
F8 = mybir.dt.float8e4
F32 = mybir.dt.float32
AF = mybir.ActivationFunctionType
ALU = mybir.AluOpType
DR = mybir.MatmulPerfMode.DoubleRow

_CACHE = {}


def _build(collectives: bool = True):
    nc = bacc.Bacc("TRN2", target_bir_lowering=False, debug=False, num_devices=CORES)

    ftT_d = nc.dram_tensor("ftT", [D_IN, M_LOC], HALF, kind="ExternalInput")
    wb_d = nc.dram_tensor("w_both", [D_IN, 2 * D_H], HALF, kind="ExternalInput")
    w2lg_d = nc.dram_tensor("w2lg", [2, D_H, D_O], HALF, kind="ExternalInput")
    c8_d = nc.dram_tensor("c8", [N, M_LOC], F8, kind="ExternalInput")
    q8_d = nc.dram_tensor("q8", [N, M_LOC], F8, kind="ExternalInput")
    # biases packed [128, 8] f32: 0-1 b1, 2-3 b1g_eff, 4 b2, 5 b2g, 6 b_c (rows 0-7)
    bias_d = nc.dram_tensor("biases", [128, 8], F32, kind="ExternalInput")
    # wadc packed [128, 10] fp16: 0 wad_L, 1 wad_G, 2-9 W_c
    wadc_d = nc.dram_tensor("wadc", [128, 10], HALF, kind="ExternalInput")
    nvec_d = nc.dram_tensor("nvec", [128, SC], F32, kind="ExternalInput")
    nrow_d = nc.dram_tensor("nrow", [1, M_LOC], HALF, kind="ExternalInput")
    out_d = nc.dram_tensor("outT", [N_CLS, M_LOC], F32, kind="ExternalOutput")

    ftT_v = ftT_d[:].rearrange("(kc p) s -> p kc s", p=128)
    c8_v = c8_d[:].rearrange("(q pair p) m -> p q pair m", p=128, pair=2)
    q8_v = q8_d[:].rearrange("(q pair p) m -> p q pair m", p=128, pair=2)

    with tile.TileContext(nc) as tc:
        with (
            tc.tile_pool(name="const", bufs=1) as cpool,
            tc.tile_pool(name="cq", bufs=1) as cq_pool,
            tc.tile_pool(name="dram", bufs=1, space="DRAM") as dram,
        ):
            # ---- constants (gpsimd DMA queue) ----
            bias_s = cpool.tile([128, 8], F32, name="bias_s")
            nc.scalar.dma_start(bias_s[:], bias_d[:])
            wadc_s = cpool.tile([128, 10], HALF, name="wadc_s")
            nc.scalar.dma_start(wadc_s[:], wadc_d[:])
            nvec_s = cpool.tile([128, SC], F32, name="nvec_s")
            nc.scalar.dma_start(nvec_s[:], nvec_d[:])
            nrow_s = cpool.tile([1, M_LOC], HALF, name="nrow_s")
            nc.scalar.dma_start(nrow_s[:], nrow_d[:])
            w2_s = cpool.tile([128, 2, NB, D_O], HALF, name="w2_s")
            nc.scalar.dma_start(w2_s[:], w2lg_d[:].rearrange("b (c p) m -> p b c m", p=128))
            ones_s = cpool.tile([1, 128], HALF, name="ones_s")
            nc.gpsimd.memset(ones_s[:], 1.0)
            ones128_8 = cpool.tile([128, 1], F8, name="ones128_8")
            nc.gpsimd.memset(ones128_8[:], 1.0)
            ones8_f = cpool.tile([8, 1], F32, name="ones8_f")
            nc.gpsimd.memset(ones8_f[:], 1.0)
            # warm the sigmoid table set (relu/copy are in every set)
            sig_warm = cpool.tile([1, 8], HALF, name="sig_warm")
            nc.scalar.activation(sig_warm[:], ones_s[:1, 0:8], AF.Sigmoid)
            junk_s = cpool.tile([128, 512], HALF, name="junk_s")
            nc.gpsimd.memset(junk_s[:], 0.125)

            def warm(region, n, free=512):
                # keep the PE p-state ramp hot through data-starved gaps; the
                # first real matmul into `region` has start=True and resets it
                if os.environ.get("NOWARM"):
                    return
                for _ in range(n):
                    nc.tensor.matmul(region[:, 0:free], junk_s[:, 0:128],
                                     junk_s[:, 0:free], start=True, stop=True,
                                     skip_group_check=True)

            # ---- sync queue: weights/features then the resident C/Q slabs ----
            ft_ctx = tc.tile_pool(name="ft", bufs=2)
            ft_pool = ft_ctx.__enter__()
            wb_s = ft_pool.tile([128, KC, 2 * D_H], HALF, name="wb_s", tag="wb")
            nc.sync.dma_start(wb_s[:], wb_d[:].rearrange("(kc p) m -> p kc m", p=128))
            ft_s = ft_pool.tile([128, KC, M_LOC], HALF, name="ft_s", tag="ft")
            for fq in range(4):
                nc.sync.dma_start(ft_s[:, :, fq * 256:(fq + 1) * 256],
                                  ftT_v[:, :, fq * 256:(fq + 1) * 256])
            c_all = cq_pool.tile([128, NPAIR, 2, M_LOC], F8, name="c_all")
            q_all = cq_pool.tile([128, NPAIR, 2, M_LOC], F8, name="q_all")
            GQ = 8  # pair-tiles per group DMA
            NG = NPAIR // GQ
            for g in range(NG):
                nc.sync.dma_start(c_all[:, g * GQ:(g + 1) * GQ], c8_v[:, g * GQ:(g + 1) * GQ])
                nc.sync.dma_start(q_all[:, g * GQ:(g + 1) * GQ], q8_v[:, g * GQ:(g + 1) * GQ])

            # bounce + gathered tensors (gathers split in row-halves so the
            # first half's collective overlaps the second half's compute)
            xb_dram = [dram.tile([M_LOC // 2, 2 * (2 * D_H)], F8, name=f"xb{i}")
                       for i in range(2)]
            xg_dram = [dram.tile([N // 2, 2 * (2 * D_H)], F8, addr_space="Shared",
                                 name=f"xg{i}") for i in range(2)]
            yb_dram = [dram.tile([M_LOC // 2, 4 * D_O], F8, name=f"yb{i}")
                       for i in range(2)]
            yg_dram = [dram.tile([N // 2, 4 * D_O], F8, addr_space="Shared",
                                 name=f"yg{i}") for i in range(2)]
            csb_dram = dram.tile([1, D_O], F32, name="csb_dram")
            cs_all_dram = dram.tile([CORES, D_O], F32, addr_space="Shared", name="cs_all")
            # gather row layout: c*1024 + l*256 + pair*128 + p ; local pair l
            # maps to new pair index j = 16*(l//2) + 2*c + (l%2)
            xg_v = [t[:].rearrange("(c l pair p) col -> p c l pair col",
                                   p=128, pair=2, l=2) for t in xg_dram]
            yg_v = [t[:].rearrange("(c l pair p) col -> p c l pair col",
                                   p=128, pair=2, l=2) for t in yg_dram]

            # ===== X = feats_slab @ [W1L | W1G] (fp16), split to fp8 hi/lo =====
            # xb columns: [hi_L 256 | hi_G 256 | lo_L 256 | lo_G 256]
            nbc_s = cpool.tile([128, M_LOC], HALF, name="nbc_s")
            with (
                tc.tile_pool(name="xstage", bufs=2) as xs_pool,
                tc.tile_pool(name="ps_x", bufs=2, space="PSUM") as psx_pool,
                tc.tile_pool(name="ps_nbc", bufs=1, space="PSUM") as psn_pool,
            ):
                # n broadcast tile for the free-dim diag(n) of the L branch
                ps_nbc = psn_pool.tile([128, M_LOC], F32, name="ps_nbc")
                warm(ps_nbc, 96, free=128)
                for h in range(NH):
                    nc.tensor.matmul(ps_nbc[:, h * F2:(h + 1) * F2], ones_s[:],
                                     nrow_s[:, h * F2:(h + 1) * F2], start=True, stop=True)
                nc.scalar.activation(nbc_s[:], ps_nbc[:], AF.Copy)

                for i in range(SC):
                    psx = psx_pool.tile([128, 2 * D_H], F32, name=f"psx{i}", tag="psx")
                    for k in range(KC):
                        nc.tensor.matmul(
                            psx[:], ft_s[:, k, i * 128:(i + 1) * 128], wb_s[:, k, :],
                            start=(k == 0), stop=(k == KC - 1),
                        )
                    xst = xs_pool.tile([128, 2 * (2 * D_H)], F8, name=f"xst{i}", tag="xst")
                    nsc = nvec_s[:, i:i + 1]
                    # hi_L = fp8(n * x_L); lo_L = fp8(n * x_L - hi_L)
                    nc.scalar.activation(xst[:, 0:D_H], psx[:, 0:D_H], AF.Copy, scale=nsc)
                    nc.scalar.activation(xst[:, D_H:2 * D_H], psx[:, D_H:2 * D_H], AF.Copy)
                    nc.vector.scalar_tensor_tensor(
                        xst[:, 2 * D_H:3 * D_H], psx[:, 0:D_H], nsc, xst[:, 0:D_H],
                        op0=ALU.mult, op1=ALU.subtract)
                    nc.vector.scalar_tensor_tensor(
                        xst[:, 3 * D_H:4 * D_H], psx[:, D_H:2 * D_H], 1.0, xst[:, D_H:2 * D_H],
                        op0=ALU.mult, op1=ALU.subtract)
                    nc.gpsimd.dma_start(xb_dram[i * 128:(i + 1) * 128, :], xst[:])

            if collectives:
                nc.gpsimd.collective_compute(
                    "AllGather", ALU.bypass,
                    ins=[xb_dram.opt()], outs=[x_all_dram.opt()],
                    replica_groups=[list(range(CORES))],
                )
            else:
                nc.gpsimd.dma_start(x_all_dram[0:M_LOC, :], xb_dram[:])
            ft_ctx.__exit__(None, None, None)

            # ===== prop1: H1 = act(prop(X)) for both branches =====
            h1_ctx = tc.tile_pool(name="h1", bufs=1)
            h1_pool = h1_ctx.__enter__()
            xh_ctx = tc.tile_pool(name="xh", bufs=1)
            xh_pool = xh_ctx.__enter__()
            xh_all = xh_pool.tile([128, NPAIR, 2, 2 * D_H], F8, name="xh_all")
            for g in range(NG):
                nc.gpsimd.dma_start(xh_all[:, g * GQ:(g + 1) * GQ],
                                    x_all_v[:, g * GQ:(g + 1) * GQ, :, 0:2 * D_H])

            with tc.tile_pool(name="ps_1", bufs=1, space="PSUM") as ps1_pool:
                psum_L = [ps1_pool.tile([128, M_LOC], F32, name=f"psl{t}") for t in range(NB)]
                psum_G = [ps1_pool.tile([128, M_LOC], F32, name=f"psg{t}") for t in range(NB)]

                xtiles = {}

                def load_xq(kind, half, ph):
                    col = slice(0, 2 * D_H) if kind == "h" else slice(2 * D_H, 4 * D_H)
                    t8 = xh_pool.tile([128, 8, 2, 2 * D_H], F8,
                                      name=f"x{kind}{half}{ph}", tag="xres")
                    eng = nc.scalar if kind == "h" else nc.gpsimd
                    if os.environ.get("SAFEDMA"):
                        for pr in range(2):
                            for cc in range(8):
                                eng.dma_start(t8[:, cc, pr, :],
                                              xg_v[half][:, cc, ph, pr, col])
                    else:
                        for pr in range(2):
                            eng.dma_start(t8[:, :, pr, :], xg_v[half][:, :, ph, pr, col])
                    xtiles[(kind, half, ph)] = t8

                def xsl(kind, j):
                    return xtiles[(kind, j // 16, j % 2)], (j % 16) // 2

                def mm1(kind, q, stop=False):
                    xt, r = xsl(kind, q)
                    for t in range(NB):
                        for h in range(NH):
                            nc.tensor.matmul(
                                psum_L[t][:, h * F2:(h + 1) * F2],
                                xt[:, r, :, t * 128:(t + 1) * 128],
                                c_all[:, q, :, h * F2:(h + 1) * F2],
                                start=(kind == "h" and q == 0), stop=stop, perf_mode=DR,
                            )
                    for t in range(NB):
                        for h in range(NH):
                            nc.tensor.matmul(
                                psum_G[t][:, h * F2:(h + 1) * F2],
                                xt[:, r, :, D_H + t * 128:D_H + (t + 1) * 128],
                                q_all[:, q, :, h * F2:(h + 1) * F2],
                                start=(kind == "h" and q == 0), stop=stop, perf_mode=DR,
                            )

                # quarters in ascending consumption order; the first lo load
                # releases the held c/q bulk (its trigger is already queued)
                for half in range(2):
                    for ph in range(2):
                        load_xq("h", half, ph)
                for half in range(2):
                    for ph in range(2):
                        load_xq("l", half, ph)
                warm(psum_L[0], 200, free=128)
                for j in range(NPAIR):
                    mm1("h", j)
                for j in range(NPAIR):
                    mm1("l", j, stop=(j == NPAIR - 1))
                # activations: H1_L = relu(n_m * agg + b1), H1_G = relu(psum/N + b1g_eff)
                h1l = [h1_pool.tile([128, M_LOC], HALF, name=f"h1l{t}") for t in range(NB)]
                h1g = [h1_pool.tile([128, M_LOC], HALF, name=f"h1g{t}") for t in range(NB)]
                tmp_ctx = tc.tile_pool(name="tmp1", bufs=2)
                tmp_pool = tmp_ctx.__enter__()
                for t in range(NB):
                    tmp = tmp_pool.tile([128, M_LOC], HALF, name=f"tm{t}", tag="tm")
                    nc.vector.tensor_mul(tmp[:], psum_L[t][:], nbc_s[:])
                    nc.scalar.activation(h1l[t][:], tmp[:], AF.Relu, bias=bias_s[:, t:t + 1])
                    nc.scalar.activation(h1g[t][:], psum_G[t][:], AF.Relu,
                                         bias=bias_s[:, 2 + t:3 + t], scale=1.0 / N)
                tmp_ctx.__exit__(None, None, None)
            xh_ctx.__exit__(None, None, None)

            # ===== Y = H1 @ W2 (both branches), fp8 hi/lo split, colsum(Y_G) =====
            # yb columns: [hi_L 128 | lo_L 128 | hi_G 128 | lo_G 128]
            with (
                tc.tile_pool(name="ystage", bufs=1) as ys_pool,
                tc.tile_pool(name="ps_y", bufs=4, space="PSUM") as psy_pool,
                tc.tile_pool(name="ps_cs", bufs=1, space="PSUM") as pcs_pool,
            ):
                ps_cs = pcs_pool.tile([1, D_O], F32, name="ps_cs")
                yst_all = ys_pool.tile([128, MB, 4 * D_O], F8, name="yst_all")
                for mb in range(MB):
                    psyl = psy_pool.tile([128, D_O], F32, name=f"pyl{mb}", tag="psy")
                    psyg = psy_pool.tile([128, D_O], F32, name=f"pyg{mb}", tag="psy")
                    for t in range(NB):
                        nc.tensor.matmul(psyl[:], h1l[t][:, mb * 128:(mb + 1) * 128],
                                         w2_s[:, 0, t, :], start=(t == 0), stop=(t == NB - 1))
                    for t in range(NB):
                        nc.tensor.matmul(psyg[:], h1g[t][:, mb * 128:(mb + 1) * 128],
                                         w2_s[:, 1, t, :], start=(t == 0), stop=(t == NB - 1))
                    nsc = nvec_s[:, mb:mb + 1]
                    nc.scalar.activation(yst[:, 0:D_O], psyl[:], AF.Copy, scale=nsc)
                    nc.vector.scalar_tensor_tensor(
                        yst[:, D_O:2 * D_O], psyl[:], nsc, yst[:, 0:D_O],
                        op0=ALU.mult, op1=ALU.subtract)
                    nc.scalar.activation(yst[:, 2 * D_O:3 * D_O], psyg[:], AF.Copy)
                    nc.vector.scalar_tensor_tensor(
                        yst[:, 3 * D_O:4 * D_O], psyg[:], 1.0, yst[:, 2 * D_O:3 * D_O],
                        op0=ALU.mult, op1=ALU.subtract)
                    # local partial colsum of Y_G' (hi + lo)
                    nc.tensor.matmul(ps_cs[:], ones128_8[:], yst[:, 2 * D_O:3 * D_O],
                                     start=(mb == 0), stop=False)
                    nc.tensor.matmul(ps_cs[:], ones128_8[:], yst[:, 3 * D_O:4 * D_O],
                                     start=False, stop=(mb == MB - 1))
                    if mb % 4 == 3:
                        half = mb // 4
                        nc.gpsimd.dma_start(
                            yb_dram[half][:].rearrange("(ch p) col -> p ch col", p=128),
                            yst_all[:, mb - 3:mb + 1, :])
                        if collectives:
                            nc.gpsimd.collective_compute(
                                "AllGather", ALU.bypass,
                                ins=[yb_dram[half].opt()], outs=[yg_dram[half].opt()],
                                replica_groups=[list(range(CORES))],
                            )
                        else:
                            nc.gpsimd.dma_start(yg_dram[half][0:512, :], yb_dram[half][:])
                cs_sb = cpool.tile([1, D_O], F32, name="cs_sb")
                nc.scalar.activation(cs_sb[:], ps_cs[:], AF.Copy)
                nc.gpsimd.dma_start(csb_dram[:], cs_sb[:])
            h1_ctx.__exit__(None, None, None)

            if collectives:
                nc.gpsimd.collective_compute(
                    "AllGather", ALU.bypass,
                    ins=[csb_dram.opt()], outs=[cs_all_dram.opt()],
                    replica_groups=[list(range(CORES))],
                )
            else:
                nc.gpsimd.dma_start(cs_all_dram[0:1, :], csb_dram[:])

            # ===== prop2 (C/Q resident, Y gathered) + fused epilogue =====
            with (
                tc.tile_pool(name="ys2", bufs=1) as ys2_pool,
                tc.tile_pool(name="epi", bufs=1) as e_pool,
            ):
                y_all = ys2_pool.tile([128, NPAIR, 2, 4 * D_O], F8, name="y_all_s")
                for g in range(NG):
                    nc.gpsimd.dma_start(y_all[:, g * GQ:(g + 1) * GQ],
                                        y_all_v[:, g * GQ:(g + 1) * GQ])
                cs8 = e_pool.tile([CORES, D_O], F32, name="cs8")
                nc.gpsimd.dma_start(cs8[:], cs_all_dram[:])
                bias_g2 = e_pool.tile([128, 1], F32, name="bias_g2")

                with tc.tile_pool(name="ps_b", bufs=1, space="PSUM") as psb_pool:
                    ps_b = psb_pool.tile([128, 1], F32, name="ps_b")
                    nc.tensor.matmul(ps_b[:], cs8[:], ones8_f[:], start=True, stop=True)
                    nc.vector.scalar_tensor_tensor(
                        bias_g2[:], ps_b[:], 0.5 / (YSCALE * N), bias_s[:, 5:6],
                        op0=ALU.mult, op1=ALU.add)

                hlt = e_pool.tile([128, M_LOC], HALF, name="hlt")
                hgt = e_pool.tile([128, M_LOC], HALF, name="hgt")
                with tc.tile_pool(name="ps_2", bufs=1, space="PSUM") as ps2_pool:
                    ps_HL = ps2_pool.tile([128, M_LOC], F32, name="ps_HL")
                    ps_HG = ps2_pool.tile([128, M_LOC], F32, name="ps_HG")
                    for q in range(NPAIR):
                        first, last = (q == 0), (q == NPAIR - 1)
                        for sub in range(2):
                            for h in range(NH):
                                nc.tensor.matmul(
                                    ps_HL[:, h * F2:(h + 1) * F2],
                                    y_all[:, q, :, sub * D_O:(sub + 1) * D_O],
                                    c_all[:, q, :, h * F2:(h + 1) * F2],
                                    start=(first and sub == 0), stop=(last and sub == 1),
                                    perf_mode=DR,
                                )
                        for sub in range(2):
                            for h in range(NH):
                                nc.tensor.matmul(
                                    ps_HG[:, h * F2:(h + 1) * F2],
                                    y_all[:, q, :, (2 + sub) * D_O:(3 + sub) * D_O],
                                    q_all[:, q, :, h * F2:(h + 1) * F2],
                                    start=(first and sub == 0), stop=(last and sub == 1),
                                    perf_mode=DR,
                                )
                    with tc.tile_pool(name="ps_b", bufs=1, space="PSUM") as psb_pool:
                        ps_b = psb_pool.tile([128, 1], F32, name="ps_b")
                        nc.tensor.matmul(ps_b[:], cs8[:], ones8_f[:], start=True, stop=True)
                        nc.vector.scalar_tensor_tensor(
                            bias_g2[:], ps_b[:], 0.5 / (YSCALE * N), cnst_s[:, 5:6],
                            op0=ALU.mult, op1=ALU.add)
                    tmp2_ctx = tc.tile_pool(name="tmp2", bufs=1)
                    tmp2_pool = tmp2_ctx.__enter__()
                    tmp2 = tmp2_pool.tile([128, M_LOC], HALF, name="tmp2")
                    nc.vector.tensor_mul(tmp2[:], ps_HL[:], nbc_s[:])
                    for h in range(NH):
                        sl = slice(h * F2, (h + 1) * F2)
                        nc.scalar.activation(hlt[:, sl], tmp2[:, sl], AF.Relu,
                                             bias=bias_s[:, 4:5])
                        nc.scalar.activation(hgt[:, sl], ps_HG[:, sl], AF.Relu,
                                             bias=bias_g2[:, 0:1], scale=1.0 / (YSCALE * N))
                    tmp2_ctx.__exit__(None, None, None)

                # ---- attention fusion + classifier, pipelined per m-half ----
                with tc.tile_pool(name="ps_3", bufs=1, space="PSUM") as ps3_pool:
                    ps_sd = ps3_pool.tile([1, M_LOC], F32, name="ps_sd")
                    ps_a0 = ps3_pool.tile([128, M_LOC], F32, name="ps_a0")
                    ps_out = ps3_pool.tile([N_CLS, M_LOC], F32, name="ps_out")
                    a0t = e_pool.tile([1, M_LOC], HALF, name="a0t")
                    d_sb = e_pool.tile([128, M_LOC], HALF, name="d_sb")
                    zt = e_pool.tile([128, M_LOC], HALF, name="zt")
                    out_sb = e_pool.tile([N_CLS, M_LOC], F32, name="out_sb")
                    for h in range(NH):
                        sl = slice(h * F2, (h + 1) * F2)
                        nc.tensor.matmul(ps_sd[:, sl], wadc_s[:, 0:1], hlt[:, sl], start=True, stop=False)
                        nc.tensor.matmul(ps_sd[:, sl], wadc_s[:, 1:2], hgt[:, sl], start=False, stop=True)
                        nc.scalar.activation(a0t[:, sl], ps_sd[:, sl], AF.Sigmoid)
                        nc.tensor.matmul(ps_a0[:, sl], ones_s[:], a0t[:, sl], start=True, stop=True)
                        nc.vector.tensor_sub(d_sb[:, sl], hlt[:, sl], hgt[:, sl])
                        nc.vector.tensor_mul(zt[:, sl], d_sb[:, sl], ps_a0[:, sl])
                        nc.vector.tensor_add(zt[:, sl], zt[:, sl], hgt[:, sl])
                        nc.tensor.matmul(ps_out[:, sl], wadc_s[:, 2:10], zt[:, sl], start=True, stop=True)
                        nc.vector.tensor_scalar_add(out_sb[:, sl], ps_out[:, sl], bias_s[0:N_CLS, 6:7])
                        nc.scalar.dma_start(out_d[:, sl], out_sb[:, sl])

    nc.compile()
    return nc


def _prep(inputs):
    """Host-side preprocessing: fold tao into weights, build the integer edge
    count matrix and the mean-shifted PPMI slab, shard / cast operands."""
    f32 = np.float32
    bf = np.float16
    feats = np.asarray(inputs["feats"], f32)
    norm = np.asarray(inputs["norm"], f32)
    PPMI = np.asarray(inputs["PPMI"], f32)
    src = np.asarray(inputs["src"]).astype(np.int64)
    dst = np.asarray(inputs["dst"]).astype(np.int64)

    w1L = np.asarray(inputs["w1"], f32) @ np.asarray(inputs["tao_1_L"], f32)
    w1G = np.asarray(inputs["w1g"], f32) @ np.asarray(inputs["tao_1_G"], f32)
    w2L = np.asarray(inputs["w2"], f32) @ np.asarray(inputs["tao_2_L"], f32)
    w2G = np.asarray(inputs["w2g"], f32) @ np.asarray(inputs["tao_2_G"], f32)
    W_a = np.asarray(inputs["W_a"], f32)
    W_c = np.asarray(inputs["W_c"], f32)

    nv = norm[:, 0]
    # integer edge-count matrix C[s, m] = #edges(s->m): exact in fp8e4
    C = np.zeros((N, N), f32)
    np.add.at(C, (src, dst), 1.0)
    C8 = C.astype(E4)
    # mean-shifted PPMI^T: Q = N*PPMI^T - 0.5 in fp8e4
    Q8 = (np.ascontiguousarray(PPMI.T) * np.float32(N) - np.float32(0.5)).astype(E4)
    # permute 256-row pair blocks into gather-half order: new pair j maps to
    # old pair 4*(jj//2) + 2*(j//16) + (jj%2), jj = j%16, so that each
    # half-gather of X/Y covers a contiguous range of new pair indices
    perm = [4 * ((j % 16) // 2) + 2 * (j // 16) + (j % 2) for j in range(32)]
    rowperm = np.concatenate([np.arange(256 * o, 256 * o + 256) for o in perm])
    C8 = C8[rowperm]
    Q8 = Q8[rowperm]

    # rank-1 mean correction for prop1-G, folded into the bias (host-exact)
    colsum_XG = (feats.sum(axis=0) @ w1G).astype(f32)
    b1g_eff = np.asarray(inputs["b1g"], f32) + np.float32(0.5 / N) * colsum_XG

    wad = (W_a[:, 0] - W_a[:, 1]).astype(f32)

    biases = np.zeros((128, 16), f32)
    biases[:, 0:2] = np.asarray(inputs["b1"], f32).reshape(NB, 128).T
    biases[:, 2:4] = b1g_eff.reshape(NB, 128).T
    biases[:, 4] = np.asarray(inputs["b2"], f32)
    biases[:, 5] = np.asarray(inputs["b2g"], f32)
    biases[:N_CLS, 6] = np.asarray(inputs["b_c"], f32)
    wadc = np.zeros((128, 10), f32)
    wadc[:, 0] = wad[:128]
    wadc[:, 1] = wad[128:]
    wadc[:, 2:10] = W_c

    common = {
        "w_both": np.concatenate([w1L, w1G], axis=1).astype(bf),
        "w2lg": np.stack([w2L, np.float32(YSCALE) * w2G]).astype(bf),

        "wadc": wadc.astype(bf),
    }
    in_maps = []
    for c in range(CORES):
        sel = slice(c * M_LOC, (c + 1) * M_LOC)
        m = dict(common)
        m["ftT"] = np.ascontiguousarray(feats[sel].T).astype(bf)
        m["c8"] = np.ascontiguousarray(C8[:, sel])
        m["q8"] = np.ascontiguousarray(Q8[:, sel])
        cn = biases.copy()
        cn[:, 8:16] = nv[sel].reshape(SC, 128).T
        m["cnst"] = cn
        m["nrow"] = nv[sel][None, :].astype(bf)
        in_maps.append(m)
    return in_maps


def kernel(**inputs) -> np.ndarray:
    if "nc" not in _CACHE:
        _CACHE["nc"] = _build()
    nc = _CACHE["nc"]
    in_maps = _prep(inputs)
    res = run_bass_kernel_spmd(nc, in_maps, list(range(CORES)), trace=False)
    out = np.empty((N, N_CLS), np.float32)
    for c in range(CORES):
        out[c * M_LOC:(c + 1) * M_LOC, :] = res.results[c]["outT"].T
    return out


if __name__ == "__main__":
    rng = np.random.default_rng(0)
    dummy = {
        "feats": rng.standard_normal((N, D_IN)).astype(np.float32),
        "norm": rng.random((N, 1)).astype(np.float32),
        "tao_1_L": rng.standard_normal((D_H, D_H)).astype(np.float32) / 16,
        "tao_2_L": rng.standard_normal((D_O, D_O)).astype(np.float32) / 11,
        "tao_1_G": rng.standard_normal((D_H, D_H)).astype(np.float32) / 16,
        "tao_2_G": rng.standard_normal((D_O, D_O)).astype(np.float32) / 11,
        "PPMI": rng.random((N, N)).astype(np.float32) / N,
        "w1": rng.random((D_IN, D_H)).astype(np.float32) * 0.06,
        "b1": rng.random((D_H,)).astype(np.float32) * 0.04,
        "w2": rng.random((D_H, D_O)).astype(np.float32) * 0.09,
        "b2": rng.random((D_O,)).astype(np.float32) * 0.06,
        "w1g": rng.random((D_IN, D_H)).astype(np.float32) * 0.06,
        "b1g": rng.random((D_H,)).astype(np.float32) * 0.04,
        "w2g": rng.random((D_H, D_O)).astype(np.float32) * 0.09,
        "b2g": rng.random((D_O,)).astype(np.float32) * 0.06,
        "W_a": rng.random((2 * D_O, 2)).astype(np.float32) * 0.7,
        "W_c": rng.random((D_O, N_CLS)).astype(np.float32) * 0.35,
        "b_c": rng.random((N_CLS,)).astype(np.float32) * 0.35,
        "src": rng.integers(0, N, (262144,)).astype(np.int32),
        "dst": rng.integers(0, N, (262144,)).astype(np.int32),
    }
    out = kernel(**dummy)
    print("out", out.shape, out.dtype, np.abs(out).mean())


# revision 46
# speedup vs baseline: 1.5210x; 1.0012x over previous
"""Trainium2 Bass kernel for nn_MetaLearner (dual-branch GCN + PPMI meta-learner).

Strategy (v2: fp8 DoubleRow propagation)
----------------------------------------
Host folds the edge-list GCN into dense matrices and factors out everything
that fp8 cannot represent exactly:

  local branch:  A' = diag(n) . C . diag(n), C[s,m] = #edges(s->m) is a small
                 INTEGER matrix -> exact in fp8e4. The diag(n) row scale is
                 folded into the X/Y operands (per-partition ACT scale), the
                 diag(n) column scale is applied with a DVE broadcast multiply.
  global branch: P = N*PPMI^T is uniform[0,1); mean-shift P = 0.5 + Q with
                 Q in fp8e4 and the exact rank-1 correction 0.5*colsum(X)
                 folded into the activation bias (host-computed for prop1,
                 device-computed + all-gathered for prop2).

All four N x N propagation matmuls then run as fp8 DoubleRow pairs (2 k-tiles
per instruction at 0.5 cycles/row = 4x fp16) with hi+lo fp8 splits of the
X/Y operands (quantization residual is itself fp8-encoded, so the pair of
DoubleRow passes is ~2x faster than fp16 at fp16-level accuracy).

Distribution: output rows sharded 1024/core. X = feats @ [W1L|W1G] is
computed on each core for its own row slab only, split hi/lo to fp8 and
AllGathered (8x less PE than the replicated-X baseline). C and Q column
slabs (8 MB each in fp8) stay RESIDENT in SBUF and are loaded once --
the baseline streamed PPMI twice at fp16.

Per-core timeline: X (fp16, 8 chunks) -> gather-X || load C/Q ->
prop1 L-hi, G-hi, then a shared lo pass (X-lo streamed once for both
branches) -> Y both branches + fp8 split + colsum(Y_G) -> gather-Y ->
prop2 (everything resident) -> fused attention + classifier.
"""

import os
import sys

sys.path.insert(0, "/opt/trn_rl_repo")

import numpy as np
import ml_dtypes

import concourse.bacc as bacc
import concourse.mybir as mybir
import concourse.tile as tile
from concourse.bass_utils import run_bass_kernel_spmd

N = 8192
D_IN = 512
D_H = 256
D_O = 128
N_CLS = 8
CORES = 8
M_LOC = N // CORES          # 1024 rows per core
NPAIR = N // 256            # 32 DoubleRow k-tile pairs
KC = D_IN // 128            # 4 k-chunks of input features
NB = D_H // 128             # 2 n-blocks of hidden features
F2 = 512                    # matmul free-dim slice
NH = M_LOC // F2            # 2 free-dim halves of the local rows
MB = M_LOC // 128           # 8 local row blocks
SC = 8                      # local s-chunks (X compute)
YSCALE = 256.0              # pre-scale for Y_G so fp8 stays in normal range

E4 = ml_dtypes.float8_e4m3
HALF = mybir.dt.float16
F8 = mybir.dt.float8e4
F32 = mybir.dt.float32
AF = mybir.ActivationFunctionType
ALU = mybir.AluOpType
DR = mybir.MatmulPerfMode.DoubleRow

_CACHE = {}


def _build(collectives: bool = True):
    nc = bacc.Bacc("TRN2", target_bir_lowering=False, debug=False, num_devices=CORES)

    ftT_d = nc.dram_tensor("ftT", [D_IN, M_LOC], HALF, kind="ExternalInput")
    wb_d = nc.dram_tensor("w_both", [D_IN, 2 * D_H], HALF, kind="ExternalInput")
    w2lg_d = nc.dram_tensor("w2lg", [2, D_H, D_O], HALF, kind="ExternalInput")
    c8_d = nc.dram_tensor("c8", [N, M_LOC], F8, kind="ExternalInput")
    q8_d = nc.dram_tensor("q8", [N, M_LOC], F8, kind="ExternalInput")
    # biases packed [128, 8] f32: 0-1 b1, 2-3 b1g_eff, 4 b2, 5 b2g, 6 b_c (rows 0-7)
    bias_d = nc.dram_tensor("biases", [128, 8], F32, kind="ExternalInput")
    # wadc packed [128, 10] fp16: 0 wad_L, 1 wad_G, 2-9 W_c
    wadc_d = nc.dram_tensor("wadc", [128, 10], HALF, kind="ExternalInput")
    nvec_d = nc.dram_tensor("nvec", [128, SC], F32, kind="ExternalInput")
    nrow_d = nc.dram_tensor("nrow", [1, M_LOC], HALF, kind="ExternalInput")
    out_d = nc.dram_tensor("outT", [N_CLS, M_LOC], F32, kind="ExternalOutput")

    ftT_v = ftT_d[:].rearrange("(kc p) s -> p kc s", p=128)
    c8_v = c8_d[:].rearrange("(q pair p) m -> p q pair m", p=128, pair=2)
    q8_v = q8_d[:].rearrange("(q pair p) m -> p q pair m", p=128, pair=2)

    with tile.TileContext(nc) as tc:
        with (
            tc.tile_pool(name="const", bufs=1) as cpool,
            tc.tile_pool(name="cq", bufs=1) as cq_pool,
            tc.tile_pool(name="dram", bufs=1, space="DRAM") as dram,
        ):
            # ---- constants (gpsimd DMA queue) ----
            bias_s = cpool.tile([128, 8], F32, name="bias_s")
            nc.scalar.dma_start(bias_s[:], bias_d[:])
            wadc_s = cpool.tile([128, 10], HALF, name="wadc_s")
            nc.scalar.dma_start(wadc_s[:], wadc_d[:])
            nvec_s = cpool.tile([128, SC], F32, name="nvec_s")
            nc.scalar.dma_start(nvec_s[:], nvec_d[:])
            nrow_s = cpool.tile([1, M_LOC], HALF, name="nrow_s")
            nc.scalar.dma_start(nrow_s[:], nrow_d[:])
            w2_s = cpool.tile([128, 2, NB, D_O], HALF, name="w2_s")
            nc.scalar.dma_start(w2_s[:], w2lg_d[:].rearrange("b (c p) m -> p b c m", p=128))
            ones_s = cpool.tile([1, 128], HALF, name="ones_s")
            nc.gpsimd.memset(ones_s[:], 1.0)
            ones128_8 = cpool.tile([128, 1], F8, name="ones128_8")
            nc.gpsimd.memset(ones128_8[:], 1.0)
            ones8_f = cpool.tile([8, 1], F32, name="ones8_f")
            nc.gpsimd.memset(ones8_f[:], 1.0)
            # warm the sigmoid table set (relu/copy are in every set)
            sig_warm = cpool.tile([1, 8], HALF, name="sig_warm")
            nc.scalar.activation(sig_warm[:], ones_s[:1, 0:8], AF.Sigmoid)
            junk_s = cpool.tile([128, 512], HALF, name="junk_s")
            nc.gpsimd.memset(junk_s[:], 0.125)

            def warm(region, n, free=512):
                # keep the PE p-state ramp hot through data-starved gaps; the
                # first real matmul into `region` has start=True and resets it
                if os.environ.get("NOWARM"):
                    return
                for _ in range(n):
                    nc.tensor.matmul(region[:, 0:free], junk_s[:, 0:128],
                                     junk_s[:, 0:free], start=True, stop=True,
                                     skip_group_check=True)

            # ---- sync queue: weights/features then the resident C/Q slabs ----
            ft_ctx = tc.tile_pool(name="ft", bufs=2)
            ft_pool = ft_ctx.__enter__()
            wb_s = ft_pool.tile([128, KC, 2 * D_H], HALF, name="wb_s", tag="wb")
            nc.sync.dma_start(wb_s[:], wb_d[:].rearrange("(kc p) m -> p kc m", p=128))
            ft_s = ft_pool.tile([128, KC, M_LOC], HALF, name="ft_s", tag="ft")
            for fq in range(4):
                nc.sync.dma_start(ft_s[:, :, fq * 256:(fq + 1) * 256],
                                  ftT_v[:, :, fq * 256:(fq + 1) * 256])
            c_all = cq_pool.tile([128, NPAIR, 2, M_LOC], F8, name="c_all")
            q_all = cq_pool.tile([128, NPAIR, 2, M_LOC], F8, name="q_all")
            GQ = 8  # pair-tiles per group DMA
            NG = NPAIR // GQ
            for g in range(NG):
                nc.sync.dma_start(c_all[:, g * GQ:(g + 1) * GQ], c8_v[:, g * GQ:(g + 1) * GQ])
                nc.sync.dma_start(q_all[:, g * GQ:(g + 1) * GQ], q8_v[:, g * GQ:(g + 1) * GQ])

            # bounce + gathered tensors (gathers split in row-halves so the
            # first half's collective overlaps the second half's compute)
            xb_dram = [dram.tile([M_LOC // 2, 2 * (2 * D_H)], F8, name=f"xb{i}")
                       for i in range(2)]
            xg_dram = [dram.tile([N // 2, 2 * (2 * D_H)], F8, addr_space="Shared",
                                 name=f"xg{i}") for i in range(2)]
            yb_dram = [dram.tile([M_LOC // 2, 4 * D_O], F8, name=f"yb{i}")
                       for i in range(2)]
            yg_dram = [dram.tile([N // 2, 4 * D_O], F8, addr_space="Shared",
                                 name=f"yg{i}") for i in range(2)]
            csb_dram = dram.tile([1, D_O], F32, name="csb_dram")
            cs_all_dram = dram.tile([CORES, D_O], F32, addr_space="Shared", name="cs_all")
            # gather row layout: c*1024 + l*256 + pair*128 + p ; local pair l
            # maps to new pair index j = 16*(l//2) + 2*c + (l%2)
            xg_v = [t[:].rearrange("(c l pair p) col -> p c l pair col",
                                   p=128, pair=2, l=2) for t in xg_dram]
            yg_v = [t[:].rearrange("(c l pair p) col -> p c l pair col",
                                   p=128, pair=2, l=2) for t in yg_dram]

            # ===== X = feats_slab @ [W1L | W1G] (fp16), split to fp8 hi/lo =====
            # xb columns: [hi_L 256 | hi_G 256 | lo_L 256 | lo_G 256]
            nbc_s = cpool.tile([128, M_LOC], HALF, name="nbc_s")
            with (
                tc.tile_pool(name="xstage", bufs=2) as xs_pool,
                tc.tile_pool(name="ps_x", bufs=2, space="PSUM") as psx_pool,
                tc.tile_pool(name="ps_nbc", bufs=1, space="PSUM") as psn_pool,
            ):
                # n broadcast tile for the free-dim diag(n) of the L branch
                ps_nbc = psn_pool.tile([128, M_LOC], F32, name="ps_nbc")
                warm(ps_nbc, 96, free=128)
                for h in range(NH):
                    nc.tensor.matmul(ps_nbc[:, h * F2:(h + 1) * F2], ones_s[:],
                                     nrow_s[:, h * F2:(h + 1) * F2], start=True, stop=True)
                nc.scalar.activation(nbc_s[:], ps_nbc[:], AF.Copy)

                for i in range(SC):
                    psx = psx_pool.tile([128, 2 * D_H], F32, name=f"psx{i}", tag="psx")
                    for k in range(KC):
                        nc.tensor.matmul(
                            psx[:], ft_s[:, k, i * 128:(i + 1) * 128], wb_s[:, k, :],
                            start=(k == 0), stop=(k == KC - 1),
                        )
                    xst = xs_pool.tile([128, 2 * (2 * D_H)], F8, name=f"xst{i}", tag="xst")
                    nsc = nvec_s[:, i:i + 1]
                    # hi_L = fp8(n * x_L); lo_L = fp8(n * x_L - hi_L)
                    nc.scalar.activation(xst[:, 0:D_H], psx[:, 0:D_H], AF.Copy, scale=nsc)
                    nc.scalar.activation(xst[:, D_H:2 * D_H], psx[:, D_H:2 * D_H], AF.Copy)
                    nc.vector.scalar_tensor_tensor(
                        xst[:, 2 * D_H:3 * D_H], psx[:, 0:D_H], nsc, xst[:, 0:D_H],
                        op0=ALU.mult, op1=ALU.subtract)
                    nc.vector.scalar_tensor_tensor(
                        xst[:, 3 * D_H:4 * D_H], psx[:, D_H:2 * D_H], 1.0, xst[:, D_H:2 * D_H],
                        op0=ALU.mult, op1=ALU.subtract)
                    nc.gpsimd.dma_start(xb_dram[i * 128:(i + 1) * 128, :], xst[:])

            if collectives:
                nc.gpsimd.collective_compute(
                    "AllGather", ALU.bypass,
                    ins=[xb_dram.opt()], outs=[x_all_dram.opt()],
                    replica_groups=[list(range(CORES))],
                )
            else:
                nc.gpsimd.dma_start(x_all_dram[0:M_LOC, :], xb_dram[:])
            ft_ctx.__exit__(None, None, None)

            # ===== prop1: H1 = act(prop(X)) for both branches =====
            h1_ctx = tc.tile_pool(name="h1", bufs=1)
            h1_pool = h1_ctx.__enter__()
            xh_ctx = tc.tile_pool(name="xh", bufs=1)
            xh_pool = xh_ctx.__enter__()
            xh_all = xh_pool.tile([128, NPAIR, 2, 2 * D_H], F8, name="xh_all")
            for g in range(NG):
                nc.gpsimd.dma_start(xh_all[:, g * GQ:(g + 1) * GQ],
                                    x_all_v[:, g * GQ:(g + 1) * GQ, :, 0:2 * D_H])

            with tc.tile_pool(name="ps_1", bufs=1, space="PSUM") as ps1_pool:
                psum_L = [ps1_pool.tile([128, M_LOC], F32, name=f"psl{t}") for t in range(NB)]
                psum_G = [ps1_pool.tile([128, M_LOC], F32, name=f"psg{t}") for t in range(NB)]

                xtiles = {}

                def load_xq(kind, half, ph):
                    col = slice(0, 2 * D_H) if kind == "h" else slice(2 * D_H, 4 * D_H)
                    t8 = xh_pool.tile([128, 8, 2, 2 * D_H], F8,
                                      name=f"x{kind}{half}{ph}", tag="xres")
                    eng = nc.scalar if kind == "h" else nc.gpsimd
                    if os.environ.get("SAFEDMA"):
                        for pr in range(2):
                            for cc in range(8):
                                eng.dma_start(t8[:, cc, pr, :],
                                              xg_v[half][:, cc, ph, pr, col])
                    else:
                        for pr in range(2):
                            eng.dma_start(t8[:, :, pr, :], xg_v[half][:, :, ph, pr, col])
                    xtiles[(kind, half, ph)] = t8

                def xsl(kind, j):
                    return xtiles[(kind, j // 16, j % 2)], (j % 16) // 2

                def mm1(kind, q, stop=False):
                    xt, r = xsl(kind, q)
                    for t in range(NB):
                        for h in range(NH):
                            nc.tensor.matmul(
                                psum_L[t][:, h * F2:(h + 1) * F2],
                                xt[:, r, :, t * 128:(t + 1) * 128],
                                c_all[:, q, :, h * F2:(h + 1) * F2],
                                start=(kind == "h" and q == 0), stop=stop, perf_mode=DR,
                            )
                    for t in range(NB):
                        for h in range(NH):
                            nc.tensor.matmul(
                                psum_G[t][:, h * F2:(h + 1) * F2],
                                xt[:, r, :, D_H + t * 128:D_H + (t + 1) * 128],
                                q_all[:, q, :, h * F2:(h + 1) * F2],
                                start=(kind == "h" and q == 0), stop=stop, perf_mode=DR,
                            )

                # quarters in ascending consumption order; the first lo load
                # releases the held c/q bulk (its trigger is already queued)
                for half in range(2):
                    for ph in range(2):
                        load_xq("h", half, ph)
                for half in range(2):
                    for ph in range(2):
                        load_xq("l", half, ph)
                warm(psum_L[0], 200, free=128)
                for j in range(NPAIR):
                    mm1("h", j)
                for j in range(NPAIR):
                    mm1("l", j, stop=(j == NPAIR - 1))
                # activations: H1_L = relu(n_m * agg + b1), H1_G = relu(psum/N + b1g_eff)
                h1l = [h1_pool.tile([128, M_LOC], HALF, name=f"h1l{t}") for t in range(NB)]
                h1g = [h1_pool.tile([128, M_LOC], HALF, name=f"h1g{t}") for t in range(NB)]
                tmp_ctx = tc.tile_pool(name="tmp1", bufs=2)
                tmp_pool = tmp_ctx.__enter__()
                for t in range(NB):
                    tmp = tmp_pool.tile([128, M_LOC], HALF, name=f"tm{t}", tag="tm")
                    nc.vector.tensor_mul(tmp[:], psum_L[t][:], nbc_s[:])
                    nc.scalar.activation(h1l[t][:], tmp[:], AF.Relu, bias=bias_s[:, t:t + 1])
                    nc.scalar.activation(h1g[t][:], psum_G[t][:], AF.Relu,
                                         bias=bias_s[:, 2 + t:3 + t], scale=1.0 / N)
                tmp_ctx.__exit__(None, None, None)
            xh_ctx.__exit__(None, None, None)

            # ===== Y = H1 @ W2 (both branches), fp8 hi/lo split, colsum(Y_G) =====
            # yb columns: [hi_L 128 | lo_L 128 | hi_G 128 | lo_G 128]
            with (
                tc.tile_pool(name="ystage", bufs=1) as ys_pool,
                tc.tile_pool(name="ps_y", bufs=4, space="PSUM") as psy_pool,
                tc.tile_pool(name="ps_cs", bufs=1, space="PSUM") as pcs_pool,
            ):
                ps_cs = pcs_pool.tile([1, D_O], F32, name="ps_cs")
                yst_all = ys_pool.tile([128, MB, 4 * D_O], F8, name="yst_all")
                for mb in range(MB):
                    psyl = psy_pool.tile([128, D_O], F32, name=f"pyl{mb}", tag="psy")
                    psyg = psy_pool.tile([128, D_O], F32, name=f"pyg{mb}", tag="psy")
                    for t in range(NB):
                        nc.tensor.matmul(psyl[:], h1l[t][:, mb * 128:(mb + 1) * 128],
                                         w2_s[:, 0, t, :], start=(t == 0), stop=(t == NB - 1))
                    for t in range(NB):
                        nc.tensor.matmul(psyg[:], h1g[t][:, mb * 128:(mb + 1) * 128],
                                         w2_s[:, 1, t, :], start=(t == 0), stop=(t == NB - 1))
                    nsc = nvec_s[:, mb:mb + 1]
                    nc.scalar.activation(yst[:, 0:D_O], psyl[:], AF.Copy, scale=nsc)
                    nc.vector.scalar_tensor_tensor(
                        yst[:, D_O:2 * D_O], psyl[:], nsc, yst[:, 0:D_O],
                        op0=ALU.mult, op1=ALU.subtract)
                    nc.scalar.activation(yst[:, 2 * D_O:3 * D_O], psyg[:], AF.Copy)
                    nc.vector.scalar_tensor_tensor(
                        yst[:, 3 * D_O:4 * D_O], psyg[:], 1.0, yst[:, 2 * D_O:3 * D_O],
                        op0=ALU.mult, op1=ALU.subtract)
                    # local partial colsum of Y_G' (hi + lo)
                    nc.tensor.matmul(ps_cs[:], ones128_8[:], yst[:, 2 * D_O:3 * D_O],
                                     start=(mb == 0), stop=False)
                    nc.tensor.matmul(ps_cs[:], ones128_8[:], yst[:, 3 * D_O:4 * D_O],
                                     start=False, stop=(mb == MB - 1))
                    if mb % 4 == 3:
                        half = mb // 4
                        nc.gpsimd.dma_start(
                            yb_dram[half][:].rearrange("(ch p) col -> p ch col", p=128),
                            yst_all[:, mb - 3:mb + 1, :])
                        if collectives:
                            nc.gpsimd.collective_compute(
                                "AllGather", ALU.bypass,
                                ins=[yb_dram[half].opt()], outs=[yg_dram[half].opt()],
                                replica_groups=[list(range(CORES))],
                            )
                        else:
                            nc.gpsimd.dma_start(yg_dram[half][0:512, :], yb_dram[half][:])
                cs_sb = cpool.tile([1, D_O], F32, name="cs_sb")
                nc.scalar.activation(cs_sb[:], ps_cs[:], AF.Copy)
                nc.gpsimd.dma_start(csb_dram[:], cs_sb[:])
            h1_ctx.__exit__(None, None, None)

            if collectives:
                nc.gpsimd.collective_compute(
                    "AllGather", ALU.bypass,
                    ins=[csb_dram.opt()], outs=[cs_all_dram.opt()],
                    replica_groups=[list(range(CORES))],
                )
            else:
                nc.gpsimd.dma_start(cs_all_dram[0:1, :], csb_dram[:])

            # ===== prop2 (C/Q resident, Y gathered) + fused epilogue =====
            with (
                tc.tile_pool(name="ys2", bufs=1) as ys2_pool,
                tc.tile_pool(name="epi", bufs=1) as e_pool,
            ):
                y_all = ys2_pool.tile([128, NPAIR, 2, 4 * D_O], F8, name="y_all_s")
                for g in range(NG):
                    nc.gpsimd.dma_start(y_all[:, g * GQ:(g + 1) * GQ],
                                        y_all_v[:, g * GQ:(g + 1) * GQ])
                cs8 = e_pool.tile([CORES, D_O], F32, name="cs8")
                nc.gpsimd.dma_start(cs8[:], cs_all_dram[:])
                bias_g2 = e_pool.tile([128, 1], F32, name="bias_g2")

                with tc.tile_pool(name="ps_b", bufs=1, space="PSUM") as psb_pool:
                    ps_b = psb_pool.tile([128, 1], F32, name="ps_b")
                    nc.tensor.matmul(ps_b[:], cs8[:], ones8_f[:], start=True, stop=True)
                    nc.vector.scalar_tensor_tensor(
                        bias_g2[:], ps_b[:], 0.5 / (YSCALE * N), bias_s[:, 5:6],
                        op0=ALU.mult, op1=ALU.add)

                hlt = e_pool.tile([128, M_LOC], HALF, name="hlt")
                hgt = e_pool.tile([128, M_LOC], HALF, name="hgt")
                with tc.tile_pool(name="ps_2", bufs=1, space="PSUM") as ps2_pool:
                    ps_HL = ps2_pool.tile([128, M_LOC], F32, name="ps_HL")
                    ps_HG = ps2_pool.tile([128, M_LOC], F32, name="ps_HG")
                    for q in range(NPAIR):
                        first, last = (q == 0), (q == NPAIR - 1)
                        for sub in range(2):
                            for h in range(NH):
                                nc.tensor.matmul(
                                    ps_HL[:, h * F2:(h + 1) * F2],
                                    y_all[:, q, :, sub * D_O:(sub + 1) * D_O],
                                    c_all[:, q, :, h * F2:(h + 1) * F2],
                                    start=(first and sub == 0), stop=(last and sub == 1),
                                    perf_mode=DR,
                                )
                        for sub in range(2):
                            for h in range(NH):
                                nc.tensor.matmul(
                                    ps_HG[:, h * F2:(h + 1) * F2],
                                    y_all[:, q, :, (2 + sub) * D_O:(3 + sub) * D_O],
                                    q_all[:, q, :, h * F2:(h + 1) * F2],
                                    start=(first and sub == 0), stop=(last and sub == 1),
                                    perf_mode=DR,
                                )
                    with tc.tile_pool(name="ps_b", bufs=1, space="PSUM") as psb_pool:
                        ps_b = psb_pool.tile([128, 1], F32, name="ps_b")
                        nc.tensor.matmul(ps_b[:], cs8[:], ones8_f[:], start=True, stop=True)
                        nc.vector.scalar_tensor_tensor(
                            bias_g2[:], ps_b[:], 0.5 / (YSCALE * N), cnst_s[:, 5:6],
                            op0=ALU.mult, op1=ALU.add)
                    tmp2_ctx = tc.tile_pool(name="tmp2", bufs=1)
                    tmp2_pool = tmp2_ctx.__enter__()
                    tmp2 = tmp2_pool.tile([128, M_LOC], HALF, name="tmp2")
                    nc.vector.tensor_mul(tmp2[:], ps_HL[:], nbc_s[:])
                    for h in range(NH):
                        sl = slice(h * F2, (h + 1) * F2)
                        nc.scalar.activation(hlt[:, sl], tmp2[:, sl], AF.Relu,
                                             bias=bias_s[:, 4:5])
                        nc.scalar.activation(hgt[:, sl], ps_HG[:, sl], AF.Relu,
                                             bias=bias_g2[:, 0:1], scale=1.0 / (YSCALE * N))
                    tmp2_ctx.__exit__(None, None, None)

                # ---- attention fusion + classifier, pipelined per m-half ----
                with tc.tile_pool(name="ps_3", bufs=1, space="PSUM") as ps3_pool:
                    ps_sd = ps3_pool.tile([1, M_LOC], F32, name="ps_sd")
                    ps_a0 = ps3_pool.tile([128, M_LOC], F32, name="ps_a0")
                    ps_out = ps3_pool.tile([N_CLS, M_LOC], F32, name="ps_out")
                    a0t = e_pool.tile([1, M_LOC], HALF, name="a0t")
                    d_sb = e_pool.tile([128, M_LOC], HALF, name="d_sb")
                    zt = e_pool.tile([128, M_LOC], HALF, name="zt")
                    out_sb = e_pool.tile([N_CLS, M_LOC], F32, name="out_sb")
                    for h in range(NH):
                        sl = slice(h * F2, (h + 1) * F2)
                        nc.tensor.matmul(ps_sd[:, sl], wadc_s[:, 0:1], hlt[:, sl], start=True, stop=False)
                        nc.tensor.matmul(ps_sd[:, sl], wadc_s[:, 1:2], hgt[:, sl], start=False, stop=True)
                        nc.scalar.activation(a0t[:, sl], ps_sd[:, sl], AF.Sigmoid)
                        nc.tensor.matmul(ps_a0[:, sl], ones_s[:], a0t[:, sl], start=True, stop=True)
                        nc.vector.tensor_sub(d_sb[:, sl], hlt[:, sl], hgt[:, sl])
                        nc.vector.tensor_mul(zt[:, sl], d_sb[:, sl], ps_a0[:, sl])
                        nc.vector.tensor_add(zt[:, sl], zt[:, sl], hgt[:, sl])
                        nc.tensor.matmul(ps_out[:, sl], wadc_s[:, 2:10], zt[:, sl], start=True, stop=True)
                        nc.vector.tensor_scalar_add(out_sb[:, sl], ps_out[:, sl], bias_s[0:N_CLS, 6:7])
                        nc.scalar.dma_start(out_d[:, sl], out_sb[:, sl])

    nc.compile()
    return nc


def _prep(inputs):
    """Host-side preprocessing: fold tao into weights, build the integer edge
    count matrix and the mean-shifted PPMI slab, shard / cast operands."""
    f32 = np.float32
    bf = np.float16
    feats = np.asarray(inputs["feats"], f32)
    norm = np.asarray(inputs["norm"], f32)
    PPMI = np.asarray(inputs["PPMI"], f32)
    src = np.asarray(inputs["src"]).astype(np.int64)
    dst = np.asarray(inputs["dst"]).astype(np.int64)

    w1L = np.asarray(inputs["w1"], f32) @ np.asarray(inputs["tao_1_L"], f32)
    w1G = np.asarray(inputs["w1g"], f32) @ np.asarray(inputs["tao_1_G"], f32)
    w2L = np.asarray(inputs["w2"], f32) @ np.asarray(inputs["tao_2_L"], f32)
    w2G = np.asarray(inputs["w2g"], f32) @ np.asarray(inputs["tao_2_G"], f32)
    W_a = np.asarray(inputs["W_a"], f32)
    W_c = np.asarray(inputs["W_c"], f32)

    nv = norm[:, 0]
    # integer edge-count matrix C[s, m] = #edges(s->m): exact in fp8e4
    C = np.zeros((N, N), f32)
    np.add.at(C, (src, dst), 1.0)
    C8 = C.astype(E4)
    # mean-shifted PPMI^T: Q = N*PPMI^T - 0.5 in fp8e4
    Q8 = (np.ascontiguousarray(PPMI.T) * np.float32(N) - np.float32(0.5)).astype(E4)
    # permute 256-row pair blocks into gather-half order: new pair j maps to
    # old pair 4*(jj//2) + 2*(j//16) + (jj%2), jj = j%16, so that each
    # half-gather of X/Y covers a contiguous range of new pair indices
    perm = [4 * ((j % 16) // 2) + 2 * (j // 16) + (j % 2) for j in range(32)]
    rowperm = np.concatenate([np.arange(256 * o, 256 * o + 256) for o in perm])
    C8 = C8[rowperm]
    Q8 = Q8[rowperm]

    # rank-1 mean correction for prop1-G, folded into the bias (host-exact)
    colsum_XG = (feats.sum(axis=0) @ w1G).astype(f32)
    b1g_eff = np.asarray(inputs["b1g"], f32) + np.float32(0.5 / N) * colsum_XG

    wad = (W_a[:, 0] - W_a[:, 1]).astype(f32)

    biases = np.zeros((128, 16), f32)
    biases[:, 0:2] = np.asarray(inputs["b1"], f32).reshape(NB, 128).T
    biases[:, 2:4] = b1g_eff.reshape(NB, 128).T
    biases[:, 4] = np.asarray(inputs["b2"], f32)
    biases[:, 5] = np.asarray(inputs["b2g"], f32)
    biases[:N_CLS, 6] = np.asarray(inputs["b_c"], f32)
    wadc = np.zeros((128, 10), f32)
    wadc[:, 0] = wad[:128]
    wadc[:, 1] = wad[128:]
    wadc[:, 2:10] = W_c

    common = {
        "w_both": np.concatenate([w1L, w1G], axis=1).astype(bf),
        "w2lg": np.stack([w2L, np.float32(YSCALE) * w2G]).astype(bf),

        "wadc": wadc.astype(bf),
    }
    in_maps = []
    for c in range(CORES):
        sel = slice(c * M_LOC, (c + 1) * M_LOC)
        m = dict(common)
        m["ftT"] = np.ascontiguousarray(feats[sel].T).astype(bf)
        m["c8"] = np.ascontiguousarray(C8[:, sel])
        m["q8"] = np.ascontiguousarray(Q8[:, sel])
        cn = biases.copy()
        cn[:, 8:16] = nv[sel].reshape(SC, 128).T
        m["cnst"] = cn
        m["nrow"] = nv[sel][None, :].astype(bf)
        in_maps.append(m)
    return in_maps


def kernel(**inputs) -> np.ndarray:
    if "nc" not in _CACHE:
        _CACHE["nc"] = _build()
    nc = _CACHE["nc"]
    in_maps = _prep(inputs)
    res = run_bass_kernel_spmd(nc, in_maps, list(range(CORES)), trace=False)
    out = np.empty((N, N_CLS), np.float32)
    for c in range(CORES):
        out[c * M_LOC:(c + 1) * M_LOC, :] = res.results[c]["outT"].T
    return out


if __name__ == "__main__":
    rng = np.random.default_rng(0)
    dummy = {
        "feats": rng.standard_normal((N, D_IN)).astype(np.float32),
        "norm": rng.random((N, 1)).astype(np.float32),
        "tao_1_L": rng.standard_normal((D_H, D_H)).astype(np.float32) / 16,
        "tao_2_L": rng.standard_normal((D_O, D_O)).astype(np.float32) / 11,
        "tao_1_G": rng.standard_normal((D_H, D_H)).astype(np.float32) / 16,
        "tao_2_G": rng.standard_normal((D_O, D_O)).astype(np.float32) / 11,
        "PPMI": rng.random((N, N)).astype(np.float32) / N,
        "w1": rng.random((D_IN, D_H)).astype(np.float32) * 0.06,
        "b1": rng.random((D_H,)).astype(np.float32) * 0.04,
        "w2": rng.random((D_H, D_O)).astype(np.float32) * 0.09,
        "b2": rng.random((D_O,)).astype(np.float32) * 0.06,
        "w1g": rng.random((D_IN, D_H)).astype(np.float32) * 0.06,
        "b1g": rng.random((D_H,)).astype(np.float32) * 0.04,
        "w2g": rng.random((D_H, D_O)).astype(np.float32) * 0.09,
        "b2g": rng.random((D_O,)).astype(np.float32) * 0.06,
        "W_a": rng.random((2 * D_O, 2)).astype(np.float32) * 0.7,
        "W_c": rng.random((D_O, N_CLS)).astype(np.float32) * 0.35,
        "b_c": rng.random((N_CLS,)).astype(np.float32) * 0.35,
        "src": rng.integers(0, N, (262144,)).astype(np.int32),
        "dst": rng.integers(0, N, (262144,)).astype(np.int32),
    }
    out = kernel(**dummy)
    print("out", out.shape, out.dtype, np.abs(out).mean())
